# revision 1
# baseline (speedup 1.0000x reference)
"""Trainium2 Bass kernel for nn_DecoderBlock_Mamba (AxialDW conv + 1x1 conv +
BN + ReLU + LN + Mamba selective scan + residual).

Sharding: 8 cores = (batch b in 0..3) x (state-half sigma in {0,1}).
Each core runs the full per-image pipeline for its batch element but only 8 of
the 16 SSM states; partial y is AllReduce'd within core pairs, post-stack is
computed redundantly on both cores of a pair.

Self-contained: hardcodes all shapes; no sibling imports.
"""
import numpy as np

C = 64
DI = 128
DS = 16
DR = 4
B = 4
H = 64
W = 64
L = H * W
NS = 8            # states per core
NCORES = 8
ROW = W + 2       # padded row stride
LP = (H + 2) * ROW
NCH = 8           # L chunks of 512
CH = 512
EPS = 1e-5

_cached = {}


def _build_program(sim=False, phases=3):
    import concourse.bass as bass
    import concourse.bacc as bacc
    import concourse.mybir as mybir
    import concourse.tile as tile

    dt = mybir.dt
    f32 = dt.float32
    bf16 = dt.bfloat16
    Act = mybir.ActivationFunctionType
    Alu = mybir.AluOpType
    Axis = mybir.AxisListType

    nc = bacc.Bacc(None, target_bir_lowering=False)

    def din(name, shape, dtype=f32):
        return nc.dram_tensor(name, shape, dtype, kind="ExternalInput")

    ximgs_d = din("ximgs", [C, 5 * L], bf16)
    cf32_d = din("cf32", [128, 19])
    cbf_d = din("cbf", [128, 2948], bf16)

    out_d = nc.dram_tensor("out_f", [C, L], f32, kind="ExternalOutput")

    groups = [[0, 1], [2, 3], [4, 5], [6, 7]]

    with tile.TileContext(nc) as tc:
        with (
            tc.tile_pool(name="dram", bufs=1, space="DRAM") as dpool,
            tc.tile_pool(name="const", bufs=1) as cpool,
            tc.tile_pool(name="big", bufs=1) as bpool,
            tc.tile_pool(name="sm", bufs=2) as spool,
            tc.tile_pool(name="da", bufs=2) as dapool,
            tc.tile_pool(name="dbx", bufs=2) as dbxpool,
            tc.tile_pool(name="ps", bufs=4, space="PSUM") as ps,
            tc.tile_pool(name="psy", bufs=2, space="PSUM") as psy,
        ):
            # ---- load constants (packed: 3 DMAs total) ----
            cf = cpool.tile([128, 19], f32)
            cb = cpool.tile([128, 2948], bf16)
            nc.sync.dma_start(cf[:], cf32_d[:])
            nc.sync.dma_start(cb[:], cbf_d[:])
            bn_s = cf[0:C, 0:1]
            bn_b = cf[0:C, 1:2]
            ip_b = cf[:, 2:4]
            cd_w = cf[:, 4:8]
            cd_b = cf[:, 8:9]
            dt_b = cf[:, 9:10]
            a_sc = cf[:, 10:18]
            Dp = cf[:, 18:19]
            ident = cb[:, 0:128]
            cw = cb[0:C, 128:448]
            ip_lhsT = cb[0:C, 448:704]
            xpdt_lhsT = cb[:, 704:708]
            dt_lhsT = cb[0:DR, 708:836]
            brep_lhsT = cb[:, 836:1860]
            crep_lhsT = cb[:, 1860:2884]
            op_lhsT = cb[:, 2884:2948]

            # ---- persistent activations ----
            SEQ = bpool.tile([C, L], bf16)           # BN+ReLU output (residual)
            HN = bpool.tile([C, L], bf16)            # LN-normalized (no affine)
            XM0 = bpool.tile([DI, L + 4], bf16)      # conv1d input, data @ col 4
            ZS = bpool.tile([DI, L], bf16)           # silu(z)
            XC = bpool.tile([DI, L], bf16)
            DT = bpool.tile([DI, L], bf16)
            U = bpool.tile([DI, L], bf16)
            Hs = [bpool.tile([DI, L], bf16, name=f"H{j}", tag=f"H{j}") for j in range(NS)]
            YSUM = bpool.tile([DI, L], bf16, name="YSUM", tag="U")

            # Prime ACT's vector clock on the const DMAs so later
            # activations (limited wait slots) don't re-wait on them.
            warm = cpool.tile([128, 1], f32, tag="warm")
            nc.scalar.activation(warm[:], cf[:, 0:1], Act.Copy)
            warm2 = cpool.tile([128, 1], bf16, tag="warm2")
            nc.scalar.activation(warm2[:], cb[:, 0:1], Act.Copy)
            eps_t = cpool.tile([128, 1], f32, tag="epsl")
            nc.gpsimd.memset(eps_t[:], EPS)
            nc.vector.tensor_scalar_mul(XM0[:, 0:4], cf[:, 0:4], 0.0)

            IMGS = [bpool.tile([C, L], bf16, name=f"img{t}", tag=f"H{t}")
                    for t in range(5)]
            for t in range(5):
                nc.sync.dma_start(IMGS[t][:], ximgs_d[:, t * L:(t + 1) * L])

            # ---- front conv: 5 accumulating taps + BN + ReLU ----
            for chi in range(NCH):
                sl = slice(chi * CH, (chi + 1) * CH)
                pc = ps.tile([C, CH], f32, tag="mm")
                for tap in range(5):
                    nc.tensor.matmul(pc[:], cw[:, tap * C:(tap + 1) * C],
                                     IMGS[tap][:, sl],
                                     start=(tap == 0), stop=(tap == 4))
                nc.scalar.activation(SEQ[:, chi * CH:(chi + 1) * CH], pc[:],
                                     Act.Relu, bias=bn_b, scale=bn_s)

            # ---- LayerNorm over channels, batched 4 blocks per DVE op ----
            HN0 = bpool.tile([128, L // 2], bf16, name="HN0", tag="HN0")
            VARS = spool.tile([128, 32], f32, tag="VARS")
            NG = L // 512  # 8 groups of 4 128-token blocks
            for g in range(NG if phases >= 1 else 0):
                tps4 = ps.tile([128, 4, C], bf16, tag="mm")
                for k in range(4):
                    blk = g * 4 + k
                    nc.tensor.transpose(tps4[:, k, :],
                                        SEQ[:, blk * 128:(blk + 1) * 128],
                                        ident[0:C, 0:C])
                mu4 = spool.tile([128, 4], f32, tag="mu4")
                nc.vector.tensor_reduce(mu4[:], tps4[:], Axis.X, Alu.add)
                mun4 = spool.tile([128, 4], f32, tag="mun4")
                nc.vector.tensor_scalar_mul(mun4[:], mu4[:], 1.0 / C)
                h04 = HN0[:, g * 256:(g + 1) * 256].rearrange(
                    "p (b c) -> p b c", b=4)
                nc.vector.tensor_tensor(h04, tps4[:],
                                        mun4[:].to_broadcast((128, 4, C)),
                                        op=Alu.subtract)
                sq4 = spool.tile([128, 4, C], f32, tag="sq4")
                nc.vector.tensor_mul(sq4[:], h04, h04)
                ssq4 = spool.tile([128, 4], f32, tag="ssq4")
                nc.vector.tensor_reduce(ssq4[:], sq4[:], Axis.X, Alu.add)
                nc.vector.tensor_scalar(VARS[:, g * 4:(g + 1) * 4], ssq4[:],
                                        1.0 / C, EPS,
                                        op0=Alu.mult, op1=Alu.add)
            SQV = spool.tile([128, 32], f32, tag="SQV")
            RSTD = spool.tile([128, 32], f32, tag="RSTD")
            if phases >= 1:
                nc.scalar.activation(SQV[:], VARS[:], Act.Sqrt)
                nc.vector.reciprocal(RSTD[:], SQV[:])
            HNT = bpool.tile([128, L // 2], bf16, name="HNT", tag="HNT")
            for g in range(NG if phases >= 1 else 0):
                hnT4 = HNT[:, g * 256:(g + 1) * 256].rearrange(
                    "p (b c) -> p b c", b=4)
                nc.vector.tensor_tensor(
                    hnT4, HN0[:, g * 256:(g + 1) * 256].rearrange(
                        "p (b c) -> p b c", b=4),
                    RSTD[:, g * 4:(g + 1) * 4].to_broadcast((128, 4, C)),
                    op=Alu.mult)
                tb4 = ps.tile([C, 4, 128], bf16, tag="mm")
                for k in range(4):
                    blk = g * 4 + k
                    nc.tensor.transpose(tb4[:, k, :],
                                        HNT[:, blk * C:(blk + 1) * C],
                                        ident)
                nc.scalar.activation(HN[:, g * CH:(g + 1) * CH],
                                     tb4[:].rearrange("p a b -> p (a b)"),
                                     Act.Copy)
            # ---- in_proj ----
            for chi in range(NCH if phases >= 1.5 else 0):
                sl = slice(chi * CH, (chi + 1) * CH)
                xm_ps = ps.tile([DI, CH], f32, tag="mm")
                z_ps = ps.tile([DI, CH], f32, tag="mm")
                nc.tensor.matmul(xm_ps[:], ip_lhsT[0:C, 0:DI], HN[:, sl],
                                 start=True, stop=True)
                nc.tensor.matmul(z_ps[:], ip_lhsT[0:C, DI:2 * DI], HN[:, sl],
                                 start=True, stop=True)
                nc.scalar.activation(XM0[:, 4 + chi * CH:4 + (chi + 1) * CH],
                                     xm_ps[:], Act.Identity, bias=ip_b[:, 0:1])
                nc.scalar.activation(ZS[:, sl], z_ps[:], Act.Silu,
                                     bias=ip_b[:, 1:2])
            # ---- causal conv1d (4 taps) + silu ----
            # xc_t = sum_k w_k * xm_{t-3+k}; XM0 holds xm at col 4,
            # XM1 at col 3: tap k reads XM0[1+k:] or XM1[k:] — use whichever
            # start offset is even so bf16 ops keep 4B alignment.
            ACC1 = bpool.tile([DI, L], bf16, name="ACC1", tag="ACC1")
            ACC2 = bpool.tile([DI, L], bf16, name="ACC2", tag="ACC2")
            if phases < 2:
                nc.gpsimd.dma_start(out_d[:, 0:CH], SEQ[:, 0:CH])
            if phases >= 2:
                nc.vector.tensor_scalar_mul(ACC1[:], XM0[:, 1:1 + L], cd_w[:, 0:1])
                nc.vector.scalar_tensor_tensor(ACC2[:], XM0[:, 2:2 + L], cd_w[:, 1:2],
                                               ACC1[:], op0=Alu.mult, op1=Alu.add)
                nc.vector.scalar_tensor_tensor(ACC1[:], XM0[:, 3:3 + L], cd_w[:, 2:3],
                                               ACC2[:], op0=Alu.mult, op1=Alu.add)
                nc.vector.scalar_tensor_tensor(ACC2[:], XM0[:, 4:4 + L], cd_w[:, 3:4],
                                               ACC1[:], op0=Alu.mult, op1=Alu.add)
                nc.scalar.activation(XC[:], ACC2[:], Act.Silu, bias=cd_b)

            # ---- x_proj (dt rows) + dt_proj + softplus ----
            for chi in range(NCH if phases >= 2 else 0):
                sl = slice(chi * CH, (chi + 1) * CH)
                dtr_ps = ps.tile([DR, CH], f32, tag="mm")
                nc.tensor.matmul(dtr_ps[:], xpdt_lhsT, XC[:, sl],
                                 start=True, stop=True)
                dtr_sb = spool.tile([DR, CH], bf16, tag="dtrsb")
                nc.scalar.activation(dtr_sb[:], dtr_ps[:], Act.Copy)
                dt_ps = ps.tile([DI, CH], f32, tag="mm")
                nc.tensor.matmul(dt_ps[:], dt_lhsT, dtr_sb[:],
                                 start=True, stop=True)
                esb = spool.tile([DI, CH], f32, tag="esb")
                nc.scalar.activation(esb[:], dt_ps[:], Act.Exp, bias=dt_b)
                nc.scalar.activation(DT[:, sl], esb[:], Act.Ln, bias=1.0)
            if phases >= 2:
                nc.vector.tensor_mul(U[:], DT[:], XC[:])

            # ---- per-state: dA = exp(a_j*dt), dBx = u*B_j, scan ----
            LH = L // 2
            for half in range(2):
                for j in range(NS if phases >= 2.5 else 0):
                    hsl = slice(half * LH, (half + 1) * LH)
                    dA = dapool.tile([DI, LH], f32, tag="dA")
                    nc.scalar.activation(dA[:], DT[:, hsl], Act.Exp,
                                         scale=a_sc[:, j:j + 1])
                    dbx = dbxpool.tile([DI, LH], bf16, tag="dbx")
                    for ci in range(LH // CH):
                        sl = slice(half * LH + ci * CH,
                                   half * LH + (ci + 1) * CH)
                        lsl = slice(ci * CH, (ci + 1) * CH)
                        br = ps.tile([DI, CH], f32, tag="mm")
                        nc.tensor.matmul(br[:], brep_lhsT[:, j * DI:(j + 1) * DI],
                                         XC[:, sl], start=True, stop=True)
                        nc.vector.tensor_tensor(dbx[:, lsl], U[:, sl], br[:],
                                                op=Alu.mult)
                    init = 0.0 if half == 0 else Hs[j][:, LH - 1:LH]
                    nc.vector.tensor_tensor_scan(Hs[j][:, hsl], dA[:], dbx[:],
                                                 init, op0=Alu.mult, op1=Alu.add)

            # ---- y accumulation: y = sum_j H_j * C_j  (PE-accumulated) ----
            y_in_t = dpool.tile([DI, L], bf16, tag="yin")
            y_out_t = dpool.tile([DI, L], bf16, tag="yout")
            for chi in range(NCH if phases >= 3 else 0):
                sl = slice(chi * CH, (chi + 1) * CH)
                yps = psy.tile([DI, CH], f32, tag="yps")
                for j in range(NS):
                    cr = ps.tile([DI, CH], f32, tag="mm")
                    nc.tensor.matmul(cr[:], crep_lhsT[:, j * DI:(j + 1) * DI],
                                     XC[:, sl], start=True, stop=True)
                    tmp = spool.tile([DI, CH], bf16, tag="ymul")
                    nc.vector.tensor_tensor(tmp[:], Hs[j][:, sl], cr[:],
                                            op=Alu.mult)
                    nc.tensor.matmul(yps[:], ident, tmp[:],
                                     start=(j == 0), stop=(j == NS - 1))
                ysb = spool.tile([DI, CH], bf16, tag="ysb")
                nc.scalar.activation(ysb[:], yps[:], Act.Copy)
                nc.sync.dma_start(y_in_t[:, sl], ysb[:])

            # ---- AllReduce partial y within batch pair (2 halves) ----
            if sim or phases < 3:
                nc.sync.dma_start(y_out_t[:], y_in_t[:])
            else:
                nc.gpsimd.collective_compute(
                    "AllReduce", Alu.add, replica_groups=groups,
                    ins=[y_in_t.opt()], outs=[y_out_t.opt()])
            nc.sync.dma_start(YSUM[:], y_out_t[:])

            # ---- post: ys = (y + xc*Dp) * silu(z); out = op(ys) + seq ----
            XCD = bpool.tile([DI, L], bf16, name="XCD", tag="DT")
            YS = bpool.tile([DI, L], bf16, tag="YS")
            for hf in range(2):
                hsl2 = slice(hf * (L // 2), (hf + 1) * (L // 2))
                nc.vector.tensor_scalar_mul(XCD[:, hsl2], XC[:, hsl2], Dp)
                nc.vector.tensor_add(XCD[:, hsl2], YSUM[:, hsl2], XCD[:, hsl2])
                nc.vector.tensor_mul(YS[:, hsl2], XCD[:, hsl2], ZS[:, hsl2])
            OUT = bpool.tile([C, L], f32, name="OUT", tag="XM0")
            for chi in range(NCH):
                sl = slice(chi * CH, (chi + 1) * CH)
                op_ps = ps.tile([C, CH], f32, tag="mm")
                nc.tensor.matmul(op_ps[:], op_lhsT, YS[:, sl],
                                 start=True, stop=True)
                nc.vector.tensor_tensor(OUT[:, sl], op_ps[:], SEQ[:, sl],
                                        op=Alu.add)
                nc.sync.dma_start(out_d[:, sl], OUT[:, sl])

    nc.compile()
    return nc


def _host_precompute(inp):
    import ml_dtypes
    f = lambda k: np.asarray(inp[k], np.float32)
    bf = lambda a: np.ascontiguousarray(a.astype(ml_dtypes.bfloat16))
    w1 = f("conv_w")[:, :, 0, 0]
    wh = f("dwh_w")[:, 0, :, 0]
    ww = f("dww_w")[:, 0, 0, :]
    taps = [
        w1 * (1.0 + wh[:, 1] + ww[:, 1])[None, :],   # center
        w1 * wh[:, 0][None, :],                       # up
        w1 * wh[:, 2][None, :],                       # down
        w1 * ww[:, 0][None, :],                       # left
        w1 * ww[:, 2][None, :],                       # right
    ]
    cw = np.concatenate([t.T for t in taps], axis=1)  # [cin=64, 5*64]
    btot = f("conv_b") + w1 @ (f("dwh_b") + f("dww_b"))
    s_bn = f("bn_g") / np.sqrt(f("bn_v") + EPS)
    bn_bias = s_bn * (btot - f("bn_m")) + f("bn_b")
    ipw = f("in_proj_w")
    ip_lhsT = (ipw * f("ln_g")[None, :]).T            # [64, 256]
    ip_bias = ipw @ f("ln_b")                          # [256]
    xpw = f("x_proj_w")                                # [36, 128]
    a_full = -np.exp(np.asarray(inp["A_log"], np.float32))  # [DI, DS]

    per_sigma = []
    for sg in range(2):
        s_lo = sg * NS
        cf32 = np.zeros((128, 19), np.float32)
        cf32[:C, 0] = s_bn
        cf32[:C, 1] = bn_bias
        cf32[:, 2] = ip_bias[:DI]
        cf32[:, 3] = ip_bias[DI:]
        cf32[:, 4:8] = f("convd_w")[:, 0, :]
        cf32[:, 8] = f("convd_b")
        cf32[:, 9] = f("dt_proj_b")
        for j in range(NS):
            cf32[:, 10 + j] = a_full[:, s_lo + j]
        cf32[:, 18] = f("Dp")

        cbf = np.zeros((128, 2948), np.float32)
        cbf[:, 0:128] = np.eye(128, dtype=np.float32)
        cbf[:C, 128:448] = cw
        cbf[:C, 448:704] = ip_lhsT
        cbf[:, 704:708] = xpw[:DR].T
        cbf[:DR, 708:836] = f("dt_proj_w").T
        for j in range(NS):
            s = s_lo + j
            cbf[:, 836 + j * DI:836 + (j + 1) * DI] = xpw[DR + s][:, None]
            cbf[:, 1860 + j * DI:1860 + (j + 1) * DI] = xpw[DR + DS + s][:, None]
        cbf[:, 2884:2948] = f("out_proj_w").T
        per_sigma.append(dict(cf32=cf32, cbf=bf(cbf)))
    return {}, per_sigma


def _shift_images(xb):
    # 5 pre-shifted copies: ctr, up(reads h-1), dn(h+1), lf(w-1), rt(w+1)
    import ml_dtypes
    out = np.zeros((C, 5, H, W), np.float32)
    out[:, 0] = xb
    out[:, 1, 1:, :] = xb[:, :-1, :]
    out[:, 2, :-1, :] = xb[:, 1:, :]
    out[:, 3, :, 1:] = xb[:, :, :-1]
    out[:, 4, :, :-1] = xb[:, :, 1:]
    return np.ascontiguousarray(
        out.transpose(1, 0, 2, 3).reshape(5, C, L).transpose(1, 0, 2)
        .reshape(C, 5 * L).astype(ml_dtypes.bfloat16))


TRACE = False
LAST_EXEC_NS = None
LAST_TRACE_DIR = None


def kernel(**inputs):
    global LAST_EXEC_NS, LAST_TRACE_DIR
    from concourse.bass_utils import run_bass_kernel_spmd

    if "nc" not in _cached:
        _cached["nc"] = _build_program()
    nc = _cached["nc"]

    common, per_sigma = _host_precompute(inputs)
    x = np.asarray(inputs["x"], np.float32)
    in_maps = []
    for c in range(NCORES):
        b, sg = c // 2, c % 2
        m = dict(common)
        m.update(per_sigma[sg])
        m["ximgs"] = _shift_images(x[b])
        in_maps.append(m)

    kw = {}
    if TRACE:
        import tempfile
        LAST_TRACE_DIR = tempfile.mkdtemp(prefix="bass_trace_")
        kw = dict(trace=True, tmpdir=LAST_TRACE_DIR)
    r = run_bass_kernel_spmd(nc, in_maps, list(range(NCORES)), **kw)
    if r.exec_time_ns is not None:
        LAST_EXEC_NS = r.exec_time_ns
    res = r.results
    out = np.empty((B, C, H, W), np.float32)
    for b in range(B):
        out[b] = np.asarray(res[2 * b]["out_f"], np.float32).reshape(C, H, W)
    return out



# revision 15
# speedup vs baseline: 1.3526x; 1.3526x over previous
"""Trainium2 Bass kernel for nn_DecoderBlock_Mamba (AxialDW conv + 1x1 conv +
BN + ReLU + LN + Mamba selective scan + residual).

Sharding: 8 cores = (batch b in 0..3) x (state-half sigma in {0,1}).
Each core runs the full per-image pipeline for its batch element but only 8 of
the 16 SSM states; partial y is AllReduce'd within core pairs; the post-stack
is computed redundantly on both cores of a pair.

Structure (vs a naive port):
- front axial+1x1 conv: 5 accumulating PE taps; up/down taps are +-64 column
  offsets into a margin-padded center image; left/right host-preshifted
- LayerNorm stats via PE ones-column matmuls into packed [8,512] PSUM rows
  (no transposes); per-token rstd and mu*rstd broadcast back via DRAM DMA
- conv1d folded into in_proj: 4 PE taps with weights w_k[d]*W_x[d,c] reading
  shifted slices of the LN output (left-padded with zeros)
- dt path collapsed to one matmul (M = dt_proj_w @ x_proj_dt), softplus as
  Exp then Ln(1+x)
- B_j/C_j state rows broadcast to [128,L] bf16 SBUF via DRAM-source DMA so
  the per-state multiplies run on DVE in 2x bf16 mode (no f32 PSUM reads)
- y = sum_j H_j*C_j accumulated into a full-PSUM [128,4096] f32 tile via
  identity matmuls; out_proj + residual fused as two accumulating matmuls

Self-contained: hardcodes all shapes; no sibling imports.
"""
import numpy as np

C = 64
DI = 128
DS = 16
DR = 4
B = 4
H = 64
W = 64
L = H * W
NS = 8            # states per core
NCORES = 8
LH = L // 2
EPS = 1e-5

_cached = {}


def _build_program(sim=False):
    import concourse.bass as bass
    import concourse.bacc as bacc
    import concourse.mybir as mybir
    import concourse.tile as tile

    dt = mybir.dt
    f32 = dt.float32
    bf16 = dt.bfloat16
    Act = mybir.ActivationFunctionType
    Alu = mybir.AluOpType

    nc = bacc.Bacc(None, target_bir_lowering=False)

    def din(name, shape, dtype=f32):
        return nc.dram_tensor(name, shape, dtype, kind="ExternalInput")

    ximgs_d = din("ximgs", [C, 3 * L], bf16)      # ctr, lf, rt
    cf32_d = din("cf32", [128, 24])
    cbf_d = din("cbf", [128, 1360], bf16)

    out_d = nc.dram_tensor("out_f", [C, L], f32, kind="ExternalOutput")

    groups = [[0, 1], [2, 3], [4, 5], [6, 7]]

    with tile.TileContext(nc) as tc:
        with (
            tc.tile_pool(name="dram", bufs=1, space="DRAM") as dpool,
            tc.tile_pool(name="const", bufs=1) as cpool,
            tc.tile_pool(name="big", bufs=1) as bpool,
            tc.tile_pool(name="sm", bufs=2) as spool,
        ):
            bc_d = dpool.tile([16, L], bf16, tag="bc_scr")
            st_d = dpool.tile([8, 1024], bf16, tag="st_scr")
            yin_d = dpool.tile([DI, L], bf16, tag="y_in")
            yout_d = dpool.tile([DI, L], bf16, tag="y_out")

            # ---- constants (2 DMAs) ----
            cf = cpool.tile([128, 24], f32)
            cb = cpool.tile([128, 1360], bf16)
            nc.sync.dma_start(cf[:], cf32_d[:])
            nc.sync.dma_start(cb[:], cbf_d[:])
            bn_s = cf[0:C, 0:1]
            bn_b = cf[0:C, 1:2]
            z_b = cf[:, 2:3]
            cd_b = cf[:, 3:4]
            dt_b = cf[:, 4:5]
            a_sc = cf[:, 5:13]
            Dp = cf[:, 13:14]
            ident = cb[:, 0:128]
            cw = cb[0:C, 128:448]          # 5 front taps [64, 5*64]
            ipz = cb[0:C, 448:576]         # [64, 128] z projection
            tapw = cb[0:C, 576:1088]       # 4 conv1d-fused taps [64, 4*128]
            mdt = cb[:, 1088:1216]         # [128, 128] dt lhsT
            bcw = cb[:, 1216:1232]         # [128, 16] B/C rows lhsT
            opw = cb[:, 1232:1296]         # [128, 64] out_proj lhsT
            hots = cb[0:C, 1296:1360]      # 8 x [64, 8] LN one-hot blocks

            # warm ACT on const DMAs (wait-slot hygiene)
            warm = cpool.tile([128, 1], f32, tag="warm")
            nc.scalar.activation(warm[:], cf[:, 0:1], Act.Copy)
            warm2 = cpool.tile([128, 1], bf16, tag="warm2")
            nc.scalar.activation(warm2[:], cb[:, 0:1], Act.Copy)

            # ---- persistent activations ----
            SEQ = bpool.tile([C, L], bf16, name="SEQ", tag="SEQ")
            HN = bpool.tile([C, 8 + L], bf16, name="HN", tag="HNp")
            ZS = bpool.tile([DI, L], bf16, name="ZS", tag="ZS")
            XC = bpool.tile([DI, L], bf16, name="XC", tag="XCp")
            DT = bpool.tile([DI, L], bf16, name="DT", tag="DTp")
            U = bpool.tile([DI, L], bf16, name="U", tag="Up")

            # rotating slots, tag-aliased with early-phase dead tiles
            IMG0 = bpool.tile([C, 64 + L + 64], bf16, name="IMG0", tag="br0")
            IMGLF = bpool.tile([C, L], bf16, name="IMGLF", tag="br1")
            IMGRT = bpool.tile([C, L], bf16, name="IMGRT", tag="br2")
            SQ = bpool.tile([C, L], bf16, name="SQ", tag="cr0")
            RSTB = bpool.tile([C, L], bf16, name="RSTB", tag="cr1")
            MRSB = bpool.tile([C, L], bf16, name="MRSB", tag="cr2")
            BCSB = bpool.tile([16, L], bf16, name="BCSB", tag="h0")
            ESB = bpool.tile([DI, 2048], f32, name="ESB", tag="h1")

            BRs = [bpool.tile([DI, L], bf16, name=f"BR{k}", tag=f"br{k}")
                   for k in range(3)]
            CRs = [bpool.tile([DI, L], bf16, name=f"CR{k}", tag=f"cr{k}")
                   for k in range(3)]
            Hs = [bpool.tile([DI, L], bf16, name=f"HH{k}", tag=f"h{k}")
                  for k in range(3)]
            TMPs = [bpool.tile([DI, L], bf16, name=f"TMP{k}", tag=f"tmp{k}")
                    for k in range(2)]
            DAs = [bpool.tile([DI, 2048], f32, name=f"DA{k}", tag=f"da{k}")
                   for k in range(3)]
            DBXs = [bpool.tile([DI, 2048], bf16, name=f"DBX{k}", tag=f"dbx{k}")
                    for k in range(3)]
            YSUM = bpool.tile([DI, L], bf16, name="YSUM", tag="da0")
            XCD = bpool.tile([DI, L], bf16, name="XCD", tag="tmp0")
            YS = bpool.tile([DI, L], bf16, name="YS", tag="tmp1")
            YSB = [bpool.tile([DI, 2048], bf16, name=f"YSB{k}", tag=f"dbx{k}")
                   for k in range(2)]
            OUTC = [bpool.tile([C, 1024], f32, name=f"OUTC{k}", tag=f"da{1+k}")
                    for k in range(2)]

            # zero margins of padded center image + HN left pad
            nc.gpsimd.memset(IMG0[:, 0:64], 0.0)
            nc.gpsimd.memset(IMG0[:, 64 + L:], 0.0)
            nc.gpsimd.memset(HN[:, 0:8], 0.0)

            nc.sync.dma_start(IMG0[:, 64:64 + L], ximgs_d[:, 0:L])
            nc.sync.dma_start(IMGLF[:], ximgs_d[:, L:2 * L])
            nc.sync.dma_start(IMGRT[:], ximgs_d[:, 2 * L:3 * L])

            # ================= phase A: front conv + LN + projections ======
            with (
                tc.tile_pool(name="psA", bufs=2, space="PSUM") as psA,
                tc.tile_pool(name="psSt", bufs=1, space="PSUM") as psSt,
            ):
                mu_ps = psSt.tile([8, 512], f32, tag="mu")
                sq_ps = psSt.tile([8, 512], f32, tag="sq")
                for g in range(4):
                    pc = psA.tile([C, 1024], f32, tag="mm")
                    for s in range(2):
                        sl0 = g * 1024 + s * 512
                        osl = pc[:, s * 512:(s + 1) * 512]
                        srcs = [IMG0[:, 64 + sl0:64 + sl0 + 512],
                                IMG0[:, sl0:sl0 + 512],
                                IMG0[:, 128 + sl0:128 + sl0 + 512],
                                IMGLF[:, sl0:sl0 + 512],
                                IMGRT[:, sl0:sl0 + 512]]
                        for tap in range(5):
                            nc.tensor.matmul(osl, cw[:, tap * C:(tap + 1) * C],
                                             srcs[tap],
                                             start=(tap == 0), stop=(tap == 4))
                    gsl = slice(g * 1024, (g + 1) * 1024)
                    nc.scalar.activation(SEQ[:, gsl], pc[:], Act.Relu,
                                         bias=bn_b, scale=bn_s)
                    nc.scalar.activation(SQ[:, gsl], SEQ[:, gsl], Act.Square)
                    for s in range(2):
                        ch = g * 2 + s
                        sl = slice(ch * 512, (ch + 1) * 512)
                        hb = hots[:, ch * 8:(ch + 1) * 8]
                        nc.tensor.matmul(mu_ps[:], hb, SEQ[:, sl],
                                         start=(ch == 0), stop=(ch == 7),
                                         skip_group_check=True)
                        nc.tensor.matmul(sq_ps[:], hb, SQ[:, sl],
                                         start=(ch == 0), stop=(ch == 7),
                                         skip_group_check=True)
                # packed LN: mu=s/64, var=sq/64-mu^2, rstd=rsqrt, mrs=mu*rstd
                MU = spool.tile([8, 512], f32, tag="MU")
                nc.vector.tensor_scalar_mul(MU[:], mu_ps[:], 1.0 / C)
                MSQ = spool.tile([8, 512], f32, tag="MSQ")
                nc.vector.tensor_scalar(MSQ[:], sq_ps[:], 1.0 / C, EPS,
                                        op0=Alu.mult, op1=Alu.add)
                MU2 = spool.tile([8, 512], f32, tag="MU2")
                nc.vector.tensor_mul(MU2[:], MU[:], MU[:])
                VAR = spool.tile([8, 512], f32, tag="VAR")
                nc.vector.tensor_tensor(VAR[:], MSQ[:], MU2[:], op=Alu.subtract)
                SQV = spool.tile([8, 512], f32, tag="SQV")
                nc.scalar.activation(SQV[:], VAR[:], Act.Sqrt)
                PK = spool.tile([8, 1024], bf16, tag="PK")
                with nc.allow_low_precision(reason="rstd broadcast is bf16"):
                    nc.vector.reciprocal(PK[:, 0:512], SQV[:])
                nc.vector.tensor_tensor(PK[:, 512:1024], MU[:], PK[:, 0:512],
                                        op=Alu.mult)
                nc.sync.dma_start(st_d[:], PK[:])
                nc.gpsimd.dma_start(
                    RSTB[:].rearrange("p (a b) -> p a b", a=8),
                    st_d[:, 0:512].rearrange("a (c b) -> c a b", c=1)
                    .to_broadcast((C, 8, 512)))
                nc.gpsimd.dma_start(
                    MRSB[:].rearrange("p (a b) -> p a b", a=8),
                    st_d[:, 512:1024].rearrange("a (c b) -> c a b", c=1)
                    .to_broadcast((C, 8, 512)))
                for hf in range(2):
                    sl = slice(hf * LH, (hf + 1) * LH)
                    hsl = slice(8 + hf * LH, 8 + (hf + 1) * LH)
                    nc.vector.tensor_tensor(HN[:, hsl], SEQ[:, sl],
                                            RSTB[:, sl], op=Alu.mult)
                    nc.vector.tensor_tensor(HN[:, hsl], HN[:, hsl],
                                            MRSB[:, sl], op=Alu.subtract)

                # ---- z (silu), xc (conv-fused silu), dt (softplus) ----
                for g in range(4):
                    zp = psA.tile([DI, 1024], f32, tag="mm")
                    for s in range(2):
                        b0 = 8 + g * 1024 + s * 512
                        nc.tensor.matmul(zp[:, s * 512:(s + 1) * 512],
                                         ipz, HN[:, b0:b0 + 512],
                                         start=True, stop=True)
                    nc.scalar.activation(ZS[:, g * 1024:(g + 1) * 1024],
                                         zp[:], Act.Silu, bias=z_b)
                for g in range(4):
                    xp = psA.tile([DI, 1024], f32, tag="mm")
                    for s in range(2):
                        b0 = 8 + g * 1024 + s * 512
                        osl = xp[:, s * 512:(s + 1) * 512]
                        for k in range(4):
                            nc.tensor.matmul(
                                osl, tapw[:, k * DI:(k + 1) * DI],
                                HN[:, b0 - 3 + k:b0 - 3 + k + 512],
                                start=(k == 0), stop=(k == 3))
                    nc.scalar.activation(XC[:, g * 1024:(g + 1) * 1024],
                                         xp[:], Act.Silu, bias=cd_b)
                for g in range(4):
                    dp = psA.tile([DI, 1024], f32, tag="mm")
                    for s in range(2):
                        sl = slice(g * 1024 + s * 512, g * 1024 + (s + 1) * 512)
                        nc.tensor.matmul(dp[:, s * 512:(s + 1) * 512],
                                         mdt, XC[:, sl], start=True, stop=True)
                    esl = ESB[:, (g % 2) * 1024:(g % 2 + 1) * 1024]
                    nc.scalar.activation(esl, dp[:], Act.Exp, bias=dt_b)
                    nc.scalar.activation(DT[:, g * 1024:(g + 1) * 1024],
                                         esl, Act.Ln, bias=1.0)
                nc.vector.tensor_mul(U[:], DT[:], XC[:])
                # ---- B/C rows for this core's 8 states: [16, L] ----
                for ch in range(8):
                    bp = psA.tile([16, 512], f32, tag="bc")
                    sl = slice(ch * 512, (ch + 1) * 512)
                    nc.tensor.matmul(bp[:], bcw, XC[:, sl],
                                     start=True, stop=True)
                    nc.scalar.activation(BCSB[:, sl], bp[:], Act.Copy)
                nc.sync.dma_start(bc_d[:], BCSB[:])

            # ================= phase B: per-state scan + y accumulation ====
            with tc.tile_pool(name="psY", bufs=1, space="PSUM") as psY:
                y_ps = psY.tile([DI, L], f32, tag="y")
                nc.gpsimd.dma_start(BRs[0][:],
                                    bc_d[0:1, :].to_broadcast((DI, L)))
                nc.gpsimd.dma_start(CRs[0][:],
                                    bc_d[8:9, :].to_broadcast((DI, L)))
                nc.gpsimd.dma_start(BRs[1][:],
                                    bc_d[1:2, :].to_broadcast((DI, L)))
                for j in range(NS):
                    br = BRs[j % 3]
                    cr = CRs[j % 3]
                    hh = Hs[j % 3]
                    tmp = TMPs[j % 2]
                    if j + 2 < NS:
                        nc.gpsimd.dma_start(
                            BRs[(j + 2) % 3][:],
                            bc_d[j + 2:j + 3, :].to_broadcast((DI, L)))
                    if j + 1 < NS:
                        nc.gpsimd.dma_start(
                            CRs[(j + 1) % 3][:],
                            bc_d[9 + j:10 + j, :].to_broadcast((DI, L)))
                    for hf in range(2):
                        sl = slice(hf * LH, (hf + 1) * LH)
                        da = DAs[(2 * j + hf) % 3]
                        nc.scalar.activation(da[:], DT[:, sl], Act.Exp,
                                             scale=a_sc[:, j:j + 1])
                        dbx = DBXs[(2 * j + hf) % 3]
                        nc.vector.tensor_tensor(dbx[:], U[:, sl], br[:, sl],
                                                op=Alu.mult)
                        init = 0.0 if hf == 0 else hh[:, LH - 1:LH]
                        nc.vector.tensor_tensor_scan(hh[:, sl], da[:], dbx[:],
                                                     init, op0=Alu.mult,
                                                     op1=Alu.add)
                    for hf in range(2):
                        sl = slice(hf * LH, (hf + 1) * LH)
                        nc.vector.tensor_tensor(tmp[:, sl], hh[:, sl],
                                                cr[:, sl], op=Alu.mult)
                    for ch in range(8):
                        sl = slice(ch * 512, (ch + 1) * 512)
                        nc.tensor.matmul(y_ps[:, sl], ident, tmp[:, sl],
                                         start=(j == 0), stop=(j == NS - 1),
                                         skip_group_check=True)
                for hf in range(2):
                    sl = slice(hf * LH, (hf + 1) * LH)
                    nc.scalar.activation(YSB[hf][:], y_ps[:, sl], Act.Copy)
                    nc.sync.dma_start(yin_d[:, sl], YSB[hf][:])

            # ================= phase C: collective + post + out ============
            if sim:
                nc.sync.dma_start(yout_d[:], yin_d[:])
            else:
                nc.gpsimd.collective_compute(
                    "AllReduce", Alu.add, replica_groups=groups,
                    ins=[yin_d.opt()], outs=[yout_d.opt()])
            nc.sync.dma_start(YSUM[:], yout_d[:])

            # ys = (y + xc*Dp) * silu(z); out = op(ys) + seq (PE-fused)
            for hf in range(2):
                sl = slice(hf * LH, (hf + 1) * LH)
                nc.vector.tensor_scalar_mul(XCD[:, sl], XC[:, sl], Dp)
                nc.vector.tensor_add(XCD[:, sl], YSUM[:, sl], XCD[:, sl])
                nc.vector.tensor_mul(YS[:, sl], XCD[:, sl], ZS[:, sl])
            with tc.tile_pool(name="psC", bufs=2, space="PSUM") as psC:
                for g in range(4):
                    op_ps = psC.tile([C, 1024], f32, tag="op")
                    for s in range(2):
                        sl = slice(g * 1024 + s * 512, g * 1024 + (s + 1) * 512)
                        osl = op_ps[:, s * 512:(s + 1) * 512]
                        nc.tensor.matmul(osl, opw, YS[:, sl],
                                         start=True, stop=False,
                                         skip_group_check=True)
                        nc.tensor.matmul(osl, ident[0:C, 0:C], SEQ[:, sl],
                                         start=False, stop=True,
                                         skip_group_check=True)
                    oc = OUTC[g % 2]
                    nc.scalar.activation(oc[:], op_ps[:], Act.Copy)
                    nc.sync.dma_start(out_d[:, g * 1024:(g + 1) * 1024], oc[:])

    nc.compile()
    return nc


def _host_precompute(inp):
    import ml_dtypes
    f = lambda k: np.asarray(inp[k], np.float32)
    bf = lambda a: np.ascontiguousarray(a.astype(ml_dtypes.bfloat16))
    w1 = f("conv_w")[:, :, 0, 0]
    wh = f("dwh_w")[:, 0, :, 0]
    ww = f("dww_w")[:, 0, 0, :]
    taps = [
        w1 * (1.0 + wh[:, 1] + ww[:, 1])[None, :],   # center
        w1 * wh[:, 0][None, :],                       # up   (reads h-1)
        w1 * wh[:, 2][None, :],                       # down (reads h+1)
        w1 * ww[:, 0][None, :],                       # left
        w1 * ww[:, 2][None, :],                       # right
    ]
    cw = np.concatenate([t.T for t in taps], axis=1)  # [64, 5*64]
    btot = f("conv_b") + w1 @ (f("dwh_b") + f("dww_b"))
    s_bn = f("bn_g") / np.sqrt(f("bn_v") + EPS)
    bn_bias = s_bn * (btot - f("bn_m")) + f("bn_b")
    ipw = f("in_proj_w")
    ipw_g = ipw * f("ln_g")[None, :]                  # LN gain folded
    ip_bias = ipw @ f("ln_b")                          # LN bias folded
    cdw = f("convd_w")[:, 0, :]                        # [128, 4]
    # conv1d-fused taps: W_k[d,c] = cdw[d,k] * ipw_g_x[d,c]
    tapw = np.concatenate(
        [(cdw[:, k:k + 1] * ipw_g[:DI]).T for k in range(4)], axis=1)
    # silu bias: convd_b plus the (constant) xm bias flowing through the taps
    cd_eff = f("convd_b") + ip_bias[:DI] * cdw.sum(1)
    xpw = f("x_proj_w")                                # [36, 128]
    mdt = (f("dt_proj_w") @ xpw[:DR]).T                # [128, 128]
    a_full = -np.exp(np.asarray(inp["A_log"], np.float32))  # [DI, DS]
    # per-chunk [64, 8] blocks: ones in column ch (LN token-sum matmuls)
    hots = np.zeros((C, 64), np.float32)
    for ch in range(8):
        hots[:, ch * 8 + ch] = 1.0

    per_sigma = []
    for sg in range(2):
        s_lo = sg * NS
        cf32 = np.zeros((128, 24), np.float32)
        cf32[:C, 0] = s_bn
        cf32[:C, 1] = bn_bias
        cf32[:, 2] = ip_bias[DI:]
        cf32[:, 3] = cd_eff
        cf32[:, 4] = f("dt_proj_b")
        for j in range(NS):
            cf32[:, 5 + j] = a_full[:, s_lo + j]
        cf32[:, 13] = f("Dp")

        cbf = np.zeros((128, 1360), np.float32)
        cbf[:, 0:128] = np.eye(128, dtype=np.float32)
        cbf[:C, 128:448] = cw
        cbf[:C, 448:576] = ipw_g[DI:].T
        cbf[:C, 576:1088] = tapw
        cbf[:, 1088:1216] = mdt
        bc_rows = np.concatenate([xpw[DR + s_lo:DR + s_lo + NS],
                                  xpw[DR + DS + s_lo:DR + DS + s_lo + NS]],
                                 axis=0)                # [16, 128]
        cbf[:, 1216:1232] = bc_rows.T
        cbf[:, 1232:1296] = f("out_proj_w").T
        cbf[:C, 1296:1360] = hots
        per_sigma.append(dict(cf32=cf32, cbf=bf(cbf)))
    return per_sigma


def _pack_images(xb):
    # 3 copies: center, left-shift source (reads w-1), right-shift (w+1)
    import ml_dtypes
    out = np.zeros((C, 3, H, W), np.float32)
    out[:, 0] = xb
    out[:, 1, :, 1:] = xb[:, :, :-1]
    out[:, 2, :, :-1] = xb[:, :, 1:]
    return np.ascontiguousarray(
        out.reshape(C, 3 * L).astype(ml_dtypes.bfloat16))


TRACE = False
LAST_EXEC_NS = None
LAST_TRACE_DIR = None


def kernel(**inputs):
    global LAST_EXEC_NS, LAST_TRACE_DIR
    from concourse.bass_utils import run_bass_kernel_spmd

    if "nc" not in _cached:
        _cached["nc"] = _build_program()
    nc = _cached["nc"]

    per_sigma = _host_precompute(inputs)
    x = np.asarray(inputs["x"], np.float32)
    in_maps = []
    for c in range(NCORES):
        b, sg = c // 2, c % 2
        m = dict(per_sigma[sg])
        m["ximgs"] = _pack_images(x[b])
        in_maps.append(m)

    kw = {}
    if TRACE:
        import tempfile
        LAST_TRACE_DIR = tempfile.mkdtemp(prefix="bass_trace_")
        kw = dict(trace=True, tmpdir=LAST_TRACE_DIR)
    r = run_bass_kernel_spmd(nc, in_maps, list(range(NCORES)), **kw)
    if r.exec_time_ns is not None:
        LAST_EXEC_NS = r.exec_time_ns
    res = r.results
    out = np.empty((B, C, H, W), np.float32)
    for b in range(B):
        out[b] = np.asarray(res[2 * b]["out_f"], np.float32).reshape(C, H, W)
    return out


# revision 37
# speedup vs baseline: 1.5596x; 1.1530x over previous
"""Trainium2 Bass kernel for nn_DecoderBlock_Mamba (AxialDW conv + 1x1 conv +
BN + ReLU + LN + Mamba selective scan + residual).

Sharding: 8 cores = (batch b in 0..3) x (state-half sigma in {0,1}).
Each core runs the full per-image pipeline for its batch element but only 8
of the 16 SSM states; partial y is AllReduce'd (per L-half) within core
pairs; the post-stack is computed redundantly on both cores of a pair.

Structure:
- everything is pipelined by L-halves front-to-back: image DMAs, the front
  conv (3 partition-paired PE taps over host-pre-shifted images, BN+ReLU in
  ACT, squares on DVE), packed LayerNorm stats (one paired PE matmul per
  chunk into [16,512] PSUM rows), rstd/-mu*rstd DRAM broadcasts, hn, the
  conv1d-fused xc taps (2 partition-paired PE matmuls over a duplicated
  LN output), the collapsed dt matmul + Exp/Ln softplus, and the B_j/C_j
  row broadcasts
- states 0-2 (first half) are emitted between the two phase halves so the
  DVE scan pipeline starts while the second half is still projecting
- per-state multiplies read bf16 SBUF broadcasts (DVE 2x mode); y =
  sum_j H_j*C_j accumulates into a full-PSUM [128,4096] f32 tile via
  identity matmuls
- z projection runs in the tail (PSUM is free then) overlapped with the
  per-half AllReduce + post + fused out_proj/residual matmuls

Self-contained: hardcodes all shapes; no sibling imports.
"""
import numpy as np

C = 64
DI = 128
DS = 16
DR = 4
B = 4
H = 64
W = 64
L = H * W
NS = 8            # states per core
NCORES = 8
LH = L // 2
IW = 64 + L + 64  # padded image width
EPS = 1e-5

_cached = {}


def _build_program(sim=False):
    import concourse.bass as bass
    import concourse.bacc as bacc
    import concourse.mybir as mybir
    import concourse.tile as tile

    dt = mybir.dt
    f32 = dt.float32
    bf16 = dt.bfloat16
    Act = mybir.ActivationFunctionType
    Alu = mybir.AluOpType

    nc = bacc.Bacc(None, target_bir_lowering=False)

    ximgs_d = nc.dram_tensor("ximgs", [128, 3 * IW], bf16,
                             kind="ExternalInput")
    cf32_d = nc.dram_tensor("cf32", [128, 24], f32, kind="ExternalInput")
    cbf_d = nc.dram_tensor("cbf", [128, 1040], bf16, kind="ExternalInput")
    out_d = nc.dram_tensor("out_f", [C, L], f32, kind="ExternalOutput")

    groups = [[0, 1], [2, 3], [4, 5], [6, 7]]

    with tile.TileContext(nc) as tc:
        with (
            tc.tile_pool(name="dram", bufs=1, space="DRAM") as dpool,
            tc.tile_pool(name="const", bufs=1) as cpool,
            tc.tile_pool(name="big", bufs=1) as bpool,
            tc.tile_pool(name="sm", bufs=1) as spool,
        ):
            bc_d = dpool.tile([16, L], bf16, tag="bc_scr")
            st_d = dpool.tile([8, 1024], bf16, tag="st_scr")
            yin_d = [dpool.tile([DI, LH], bf16, name=f"y_in{k}",
                                tag=f"y_in{k}") for k in range(2)]
            yout_d = [dpool.tile([DI, LH], bf16, name=f"y_out{k}",
                                 tag=f"y_out{k}") for k in range(2)]

            cf = cpool.tile([128, 24], f32)
            cb = cpool.tile([128, 1040], bf16)
            nc.sync.dma_start(cf[:], cf32_d[:])
            nc.sync.dma_start(cb[:], cbf_d[:])
            bn_s = cf[:, 0:1]
            bn_b = cf[:, 1:2]
            z_b = cf[:, 2:3]
            cd_b = cf[:, 3:4]
            dt_b = cf[:, 4:5]
            a_sc = cf[:, 5:13]
            Dp = cf[:, 13:14]
            ident = cb[:, 0:128]
            cwp = cb[:, 128:320]           # 3 paired front taps [128, 3*64]
            ipz = cb[0:C, 320:448]         # [64, 128] z projection
            tapp = cb[:, 448:704]          # 2 paired conv1d taps [128, 2*128]
            mdt = cb[:, 704:832]           # [128, 128] dt lhsT
            bcw = cb[:, 832:848]           # [128, 16] B/C rows lhsT
            opw = cb[:, 848:912]           # [128, 64] out_proj lhsT
            hotq = cb[:, 912:976]          # 8 x ([128,4] mu + [128,4] sq)

            warm = cpool.tile([128, 1], f32, tag="warm")
            nc.scalar.activation(warm[:], cf[:, 0:1], Act.Copy)
            warm2 = cpool.tile([128, 1], bf16, tag="warm2")
            nc.scalar.activation(warm2[:], cb[:, 0:1], Act.Copy)

            # ---- persistent activations ----
            SEQSQ = bpool.tile([128, L], bf16, name="SEQSQ", tag="SEQSQ")
            SEQ = SEQSQ[0:C, :]
            SQ = SEQSQ[C:128, :]
            HNP = bpool.tile([128, 12 + L], bf16, name="HNP", tag="HNP")
            ZS = bpool.tile([DI, L], bf16, name="ZS", tag="ZSp")
            XC = bpool.tile([DI, L], bf16, name="XC", tag="XCp")
            DT = bpool.tile([DI, L], bf16, name="DT", tag="DTp")
            U = bpool.tile([DI, L], bf16, name="U", tag="Up")
            XCD = bpool.tile([DI, L], bf16, name="XCD", tag="xcd")

            # rotating full tiles whose halves serve as per-half slots;
            # tag-aliased with early-phase dead tiles
            P1 = bpool.tile([128, IW], bf16, name="P1", tag="br0")
            P2 = bpool.tile([128, IW], bf16, name="P2", tag="br1")
            P3 = bpool.tile([128, IW], bf16, name="P3", tag="br2")
            RSTB = bpool.tile([C, L], bf16, name="RSTB", tag="cr0")
            MRSB = bpool.tile([C, L], bf16, name="MRSB", tag="cr1")
            BCSB = bpool.tile([16, L], bf16, name="BCSB", tag="cr2")
            ESB = bpool.tile([DI, 2048], f32, name="ESB", tag="tmp0")

            BRF = [bpool.tile([DI, L], bf16, name=f"BR{k}", tag=f"br{k}")
                   for k in range(3)]
            CRF = [bpool.tile([DI, L], bf16, name=f"CR{k}", tag=f"cr{k}")
                   for k in range(3)]
            Hs = [bpool.tile([DI, L], bf16, name=f"HH{k}", tag=f"h{k}")
                  for k in range(3)]
            TMPs = [bpool.tile([DI, L], bf16, name=f"TMP{k}", tag=f"tmp{k}")
                    for k in range(2)]
            DAs = [bpool.tile([DI, 2048], f32, name=f"DA{k}", tag=f"da{k}")
                   for k in range(4)]
            DBXs = [bpool.tile([DI, 2048], bf16, name=f"DBX{k}",
                               tag=f"dbx{k}") for k in range(4)]
            YSUM = [bpool.tile([DI, 2048], bf16, name=f"YSUM{k}",
                               tag=f"da{k}") for k in range(2)]
            YS = [bpool.tile([DI, 2048], bf16, name=f"YS{k}", tag=f"dbx{k}")
                  for k in range(2)]
            YSB = [bpool.tile([DI, 2048], bf16, name=f"YSB{k}",
                              tag=["tmp1", "da2"][k]) for k in range(2)]
            OUTC = [bpool.tile([C, 1024], f32, name="OUTC0", tag="dbx2"),
                    bpool.tile([C, 1024], f32, name="OUTC1", tag="dbx3")]

            def br_slot(j, hf):
                m = (2 * j + hf) % 6
                return BRF[m // 2][:, (m % 2) * LH:(m % 2 + 1) * LH]

            def cr_slot(j, hf):
                m = (2 * j + hf) % 6
                return CRF[m // 2][:, (m % 2) * LH:(m % 2 + 1) * LH]

            def bcast(dst, row, hf):
                src = bc_d[row:row + 1, hf * LH:(hf + 1) * LH]
                nc.gpsimd.dma_start(dst[:], src.to_broadcast((DI, LH)))

            nc.gpsimd.memset(HNP[:, 0:12], 0.0)

            # image DMAs, first-half columns first
            IWH = 64 + LH + 64
            for hf in range(2):
                c0 = IWH if hf else 0
                c1 = IW if hf else IWH
                for t, P in enumerate([P1, P2, P3]):
                    nc.sync.dma_start(P[:, c0:c1],
                                      ximgs_d[:, t * IW + c0:t * IW + c1])

            with (
                tc.tile_pool(name="psA", bufs=3, space="PSUM") as psA,
                tc.tile_pool(name="psSt", bufs=1, space="PSUM") as psSt,
                tc.tile_pool(name="psB", bufs=1, space="PSUM") as psB,
            ):
                st_ps = [psSt.tile([4, 1024], f32, name=f"st{h}",
                                   tag=f"st{h}") for h in range(2)]

                def front_half(hf):
                    for gg in range(4):
                        ch = hf * 4 + gg
                        pc = psA.tile([C, 512], f32, tag="mm")
                        sl0 = ch * 512
                        for tp in range(3):
                            nc.tensor.matmul(
                                pc[:], cwp[:, tp * C:(tp + 1) * C],
                                [P1, P2, P3][tp][:, 64 + sl0:64 + sl0 + 512],
                                start=(tp == 0), stop=(tp == 2))
                        gsl = slice(ch * 512, (ch + 1) * 512)
                        nc.scalar.activation(SEQ[:, gsl], pc[:], Act.Relu,
                                             bias=bn_b[0:C], scale=bn_s[0:C])
                        nc.vector.tensor_mul(SQ[:, gsl], SEQ[:, gsl],
                                             SEQ[:, gsl])
                        # token sums -> cols 0-511, sq sums -> cols 512-1023,
                        # row = ch%4 (keeps all reads partition-0-aligned)
                        nc.tensor.matmul(st_ps[hf][:, 0:512],
                                         hotq[:, ch * 8:ch * 8 + 4],
                                         SEQSQ[:, gsl],
                                         start=(gg == 0), stop=(gg == 3),
                                         skip_group_check=True)
                        nc.tensor.matmul(st_ps[hf][:, 512:1024],
                                         hotq[:, ch * 8 + 4:ch * 8 + 8],
                                         SEQSQ[:, gsl],
                                         start=(gg == 0), stop=(gg == 3),
                                         skip_group_check=True)
                    # LN smalls for this half ([4, 512], partition offset 0)
                    MU = spool.tile([4, 512], f32, name=f"MU{hf}", tag="MU")
                    MSQ = spool.tile([4, 512], f32, name=f"MSQ{hf}",
                                     tag="MSQ")
                    MU2 = spool.tile([4, 512], f32, name=f"MU2{hf}",
                                     tag="MU2")
                    PK = spool.tile([4, 1024], bf16, name=f"PK{hf}",
                                    tag="PK")
                    nc.vector.tensor_scalar_mul(MU[:], st_ps[hf][:, 0:512],
                                                1.0 / C)
                    nc.vector.tensor_scalar(MSQ[:], st_ps[hf][:, 512:1024],
                                            1.0 / C, EPS,
                                            op0=Alu.mult, op1=Alu.add)
                    nc.vector.tensor_mul(MU2[:], MU[:], MU[:])
                    nc.vector.tensor_tensor(MSQ[:], MSQ[:], MU2[:],
                                            op=Alu.subtract)
                    nc.scalar.activation(MU2[:], MSQ[:], Act.Sqrt)
                    with nc.allow_low_precision(reason="bf16 rstd bcast"):
                        nc.vector.reciprocal(PK[:, 0:512], MU2[:])
                    nc.vector.tensor_tensor(PK[:, 512:1024], MU[:],
                                            PK[:, 0:512], op=Alu.mult)
                    rsl = slice(hf * 4, hf * 4 + 4)
                    nc.sync.dma_start(st_d[rsl, :], PK[:])
                    hsl = slice(hf * LH, (hf + 1) * LH)
                    nc.gpsimd.dma_start(
                        RSTB[:, hsl].rearrange("p (a b) -> p a b", a=4),
                        st_d[rsl, 0:512].rearrange("a (c b) -> c a b", c=1)
                        .to_broadcast((C, 4, 512)))
                    nc.gpsimd.dma_start(
                        MRSB[:, hsl].rearrange("p (a b) -> p a b", a=4),
                        st_d[rsl, 512:1024].rearrange("a (c b) -> c a b", c=1)
                        .to_broadcast((C, 4, 512)))
                    psl = slice(8 + hf * LH, 8 + (hf + 1) * LH)
                    nc.vector.tensor_tensor(HNP[0:C, psl], SEQ[:, hsl],
                                            RSTB[:, hsl], op=Alu.mult)
                    nc.vector.tensor_tensor(HNP[0:C, psl], HNP[0:C, psl],
                                            MRSB[:, hsl], op=Alu.subtract)
                    # duplicated upper copy, shifted one column left
                    nc.sync.dma_start(HNP[C:128, 7 + hf * LH:7 + (hf + 1) * LH],
                                      HNP[0:C, 8 + hf * LH:8 + (hf + 1) * LH])

                def proj_half(hf):
                    hsl = slice(hf * LH, (hf + 1) * LH)
                    for gg in range(4):
                        ch = hf * 4 + gg
                        xp = psA.tile([DI, 512], f32, tag="mm")
                        b0 = 8 + ch * 512
                        nc.tensor.matmul(xp[:], tapp[:, 0:DI],
                                         HNP[:, b0 - 3:b0 - 3 + 512],
                                         start=True, stop=False)
                        nc.tensor.matmul(xp[:], tapp[:, DI:2 * DI],
                                         HNP[:, b0 - 1:b0 - 1 + 512],
                                         start=False, stop=True)
                        nc.scalar.activation(XC[:, ch * 512:(ch + 1) * 512],
                                             xp[:], Act.Silu, bias=cd_b)
                    for ch in range(hf * 4, hf * 4 + 4):
                        bp = psB.tile([16, 512], f32, tag="bc")
                        sl = slice(ch * 512, (ch + 1) * 512)
                        nc.tensor.matmul(bp[:], bcw, XC[:, sl],
                                         start=True, stop=True)
                        nc.vector.tensor_copy(BCSB[:, sl], bp[:])
                    nc.sync.dma_start(bc_d[:, hsl], BCSB[:, hsl])
                    for gg in range(4):
                        ch = hf * 4 + gg
                        dp = psA.tile([DI, 512], f32, tag="mm")
                        sl = slice(ch * 512, (ch + 1) * 512)
                        nc.tensor.matmul(dp[:], mdt, XC[:, sl],
                                         start=True, stop=True)
                        esl = ESB[:, gg * 512:(gg + 1) * 512]
                        nc.scalar.activation(esl, dp[:], Act.Exp, bias=dt_b)
                        nc.scalar.activation(DT[:, sl], esl, Act.Ln, bias=1.0)
                    nc.vector.tensor_tensor(U[:, hsl], DT[:, hsl], XC[:, hsl],
                                            op=Alu.mult)
                    nc.vector.tensor_scalar_mul(XCD[:, hsl], XC[:, hsl], Dp)

                def state_half(j, hf, slot=None):
                    sl = slice(hf * LH, (hf + 1) * LH)
                    hh = Hs[j % 3]
                    if slot is None:
                        slot = (2 * j + hf) % 4
                    da = DAs[slot]
                    nc.scalar.activation(da[:], DT[:, sl], Act.Exp,
                                         scale=a_sc[:, j:j + 1])
                    dbx = DBXs[slot]
                    nc.vector.tensor_tensor(dbx[:], U[:, sl],
                                            br_slot(j, hf), op=Alu.mult)
                    init = 0.0 if hf == 0 else hh[:, LH - 1:LH]
                    nc.vector.tensor_tensor_scan(hh[:, sl], da[:], dbx[:],
                                                 init, op0=Alu.mult,
                                                 op1=Alu.add)

                front_half(0)
                front_half(1)
                proj_half(0)
                bcast(br_slot(0, 0), 0, 0)
                bcast(cr_slot(0, 0), 8, 0)
                bcast(br_slot(1, 0), 1, 0)
                bcast(cr_slot(1, 0), 9, 0)
                bcast(br_slot(2, 0), 2, 0)
                state_half(0, 0)
                state_half(1, 0)
                state_half(2, 0, slot=3)
                proj_half(1)
                bcast(br_slot(0, 1), 0, 1)
                bcast(cr_slot(0, 1), 8, 1)

            # ================= state loop =================================
            with tc.tile_pool(name="psY", bufs=1, space="PSUM") as psY:
                y_ps = psY.tile([DI, L], f32, tag="y")
                for j in range(NS):
                    hh = Hs[j % 3]
                    tmp = TMPs[j % 2]
                    if j + 1 < NS:
                        bcast(br_slot(j + 1, 1), j + 1, 1)
                        bcast(cr_slot(j + 1, 1), 8 + j + 1, 1)
                    if j + 2 < NS:
                        bcast(cr_slot(j + 2, 0), 8 + j + 2, 0)
                    if j + 3 < NS:
                        bcast(br_slot(j + 3, 0), j + 3, 0)
                    if j >= 3:
                        state_half(j, 0)
                    state_half(j, 1)
                    for hf in range(2):
                        sl = slice(hf * LH, (hf + 1) * LH)
                        nc.vector.tensor_tensor(tmp[:, sl], hh[:, sl],
                                                cr_slot(j, hf), op=Alu.mult)
                        for ch in range(4):
                            psl = slice(hf * LH + ch * 512,
                                        hf * LH + (ch + 1) * 512)
                            nc.tensor.matmul(y_ps[:, psl], ident,
                                             tmp[:, psl], start=(j == 0),
                                             stop=(j == NS - 1),
                                             skip_group_check=True)
                for hf in range(2):
                    sl = slice(hf * LH, (hf + 1) * LH)
                    nc.scalar.activation(YSB[hf][:], y_ps[:, sl], Act.Copy)
                    nc.sync.dma_start(yin_d[hf][:], YSB[hf][:])
                    if sim:
                        nc.sync.dma_start(yout_d[hf][:], yin_d[hf][:])
                    else:
                        nc.gpsimd.collective_compute(
                            "AllReduce", Alu.add, replica_groups=groups,
                            ins=[yin_d[hf].opt()],
                            outs=[yout_d[hf].opt()])

            # ====== tail: z projection + post + out, per half =============
            with tc.tile_pool(name="psC", bufs=2, space="PSUM") as psC:
                for hf in range(2):
                    sl = slice(hf * LH, (hf + 1) * LH)
                    for gg in range(2):
                        g = hf * 2 + gg
                        zp = psC.tile([DI, 1024], f32, tag="zmm")
                        for s in range(2):
                            b0 = 8 + g * 1024 + s * 512
                            nc.tensor.matmul(zp[:, s * 512:(s + 1) * 512],
                                             ipz, HNP[0:C, b0:b0 + 512],
                                             start=True, stop=True)
                        nc.scalar.activation(ZS[:, g * 1024:(g + 1) * 1024],
                                             zp[:], Act.Silu, bias=z_b)
                    nc.sync.dma_start(YSUM[hf][:], yout_d[hf][:])
                    nc.vector.tensor_add(YS[hf][:], YSUM[hf][:], XCD[:, sl])
                    nc.vector.tensor_mul(YS[hf][:], YS[hf][:], ZS[:, sl])
                    for gg in range(2):
                        op_ps = psC.tile([C, 1024], f32, tag="op")
                        for s in range(2):
                            c0 = gg * 1024 + s * 512
                            osl = op_ps[:, s * 512:(s + 1) * 512]
                            nc.tensor.matmul(osl, opw,
                                             YS[hf][:, c0:c0 + 512],
                                             start=True, stop=False,
                                             skip_group_check=True)
                            nc.tensor.matmul(osl, ident[0:C, 0:C],
                                             SEQ[:, hf * LH + c0:
                                                 hf * LH + c0 + 512],
                                             start=False, stop=True,
                                             skip_group_check=True)
                        oc = OUTC[gg]
                        nc.scalar.activation(oc[:], op_ps[:], Act.Copy)
                        nc.sync.dma_start(
                            out_d[:, hf * LH + gg * 1024:
                                  hf * LH + (gg + 1) * 1024], oc[:])

    nc.compile()
    return nc


def _host_precompute(inp):
    import ml_dtypes
    f = lambda k: np.asarray(inp[k], np.float32)
    bf = lambda a: np.ascontiguousarray(a.astype(ml_dtypes.bfloat16))
    w1 = f("conv_w")[:, :, 0, 0]
    wh = f("dwh_w")[:, 0, :, 0]
    ww = f("dww_w")[:, 0, 0, :]
    taps = [
        w1 * (1.0 + wh[:, 1] + ww[:, 1])[None, :],   # center
        w1 * wh[:, 0][None, :],                       # up   (reads h-1)
        w1 * wh[:, 2][None, :],                       # down (reads h+1)
        w1 * ww[:, 0][None, :],                       # left
        w1 * ww[:, 2][None, :],                       # right
    ]
    z64 = np.zeros((C, C), np.float32)
    cwp = np.concatenate([
        np.concatenate([taps[0].T, taps[1].T], axis=0),
        np.concatenate([taps[2].T, taps[3].T], axis=0),
        np.concatenate([taps[4].T, z64], axis=0),
    ], axis=1)                                        # [128, 3*64]
    btot = f("conv_b") + w1 @ (f("dwh_b") + f("dww_b"))
    s_bn = f("bn_g") / np.sqrt(f("bn_v") + EPS)
    bn_bias = s_bn * (btot - f("bn_m")) + f("bn_b")
    ipw = f("in_proj_w")
    ipw_g = ipw * f("ln_g")[None, :]
    ip_bias = ipw @ f("ln_b")
    cdw = f("convd_w")[:, 0, :]                       # [128, 4]
    tk = [(cdw[:, k:k + 1] * ipw_g[:DI]).T for k in range(4)]  # [64,128]
    tapp = np.concatenate([
        np.concatenate([tk[0], tk[1]], axis=0),
        np.concatenate([tk[2], tk[3]], axis=0),
    ], axis=1)                                        # [128, 2*128]
    cd_eff = f("convd_b") + ip_bias[:DI] * cdw.sum(1)
    xpw = f("x_proj_w")
    mdt = (f("dt_proj_w") @ xpw[:DR]).T               # [128, 128]
    a_full = -np.exp(np.asarray(inp["A_log"], np.float32))
    # per chunk ch: [128,4] mu block (ones col ch%4, top rows) then [128,4]
    # sq block (ones col ch%4, bottom rows)
    hotq = np.zeros((128, 64), np.float32)
    for ch in range(8):
        hotq[0:C, ch * 8 + ch % 4] = 1.0
        hotq[C:128, ch * 8 + 4 + ch % 4] = 1.0

    per_sigma = []
    for sg in range(2):
        s_lo = sg * NS
        cf32 = np.zeros((128, 24), np.float32)
        cf32[:C, 0] = s_bn
        cf32[:C, 1] = bn_bias
        cf32[:, 2] = ip_bias[DI:]
        cf32[:, 3] = cd_eff
        cf32[:, 4] = f("dt_proj_b")
        for j in range(NS):
            cf32[:, 5 + j] = a_full[:, s_lo + j]
        cf32[:, 13] = f("Dp")

        cbf = np.zeros((128, 1040), np.float32)
        cbf[:, 0:128] = np.eye(128, dtype=np.float32)
        cbf[:, 128:320] = cwp
        cbf[:C, 320:448] = ipw_g[DI:].T
        cbf[:, 448:704] = tapp
        cbf[:, 704:832] = mdt
        bc_rows = np.concatenate([xpw[DR + s_lo:DR + s_lo + NS],
                                  xpw[DR + DS + s_lo:DR + DS + s_lo + NS]],
                                 axis=0)
        cbf[:, 832:848] = bc_rows.T
        cbf[:, 848:912] = f("out_proj_w").T
        cbf[:, 912:976] = hotq
        per_sigma.append(dict(cf32=cf32, cbf=bf(cbf)))
    return per_sigma


def _pack_images(xb):
    """3 paired tiles [128, IW]: reading cols 64+t..64+t+511 yields
    (ctr[t], ctr[t-64]), (ctr[t+64], lf[t]), (rt[t], 0)."""
    import ml_dtypes
    ctr = xb.reshape(C, L)
    lf = np.zeros((C, H, W), np.float32)
    lf[:, :, 1:] = xb[:, :, :-1]
    rt = np.zeros((C, H, W), np.float32)
    rt[:, :, :-1] = xb[:, :, 1:]
    out = np.zeros((128, 3 * IW), np.float32)
    out[0:C, 64:64 + L] = ctr
    out[C:128, 128:128 + L] = ctr
    out[0:C, IW + 0:IW + L] = ctr
    out[C:128, IW + 64:IW + 64 + L] = lf.reshape(C, L)
    out[0:C, 2 * IW + 64:2 * IW + 64 + L] = rt.reshape(C, L)
    return np.ascontiguousarray(out.astype(ml_dtypes.bfloat16))


TRACE = False
LAST_EXEC_NS = None
LAST_TRACE_DIR = None


def kernel(**inputs):
    global LAST_EXEC_NS, LAST_TRACE_DIR
    from concourse.bass_utils import run_bass_kernel_spmd

    if "nc" not in _cached:
        _cached["nc"] = _build_program()
    nc = _cached["nc"]

    per_sigma = _host_precompute(inputs)
    x = np.asarray(inputs["x"], np.float32)
    in_maps = []
    for c in range(NCORES):
        b, sg = c // 2, c % 2
        m = dict(per_sigma[sg])
        m["ximgs"] = _pack_images(x[b])
        in_maps.append(m)

    kw = {}
    if TRACE:
        import tempfile
        LAST_TRACE_DIR = tempfile.mkdtemp(prefix="bass_trace_")
        kw = dict(trace=True, tmpdir=LAST_TRACE_DIR)
    r = run_bass_kernel_spmd(nc, in_maps, list(range(NCORES)), **kw)
    if r.exec_time_ns is not None:
        LAST_EXEC_NS = r.exec_time_ns
    res = r.results
    out = np.empty((B, C, H, W), np.float32)
    for b in range(B):
        out[b] = np.asarray(res[2 * b]["out_f"], np.float32).reshape(C, H, W)
    return out


# revision 47
# speedup vs baseline: 1.5804x; 1.0134x over previous
"""Trainium2 Bass kernel for nn_DecoderBlock_Mamba (AxialDW conv + 1x1 conv +
BN + ReLU + LN + Mamba selective scan + residual).

Sharding: 8 cores = (batch b in 0..3) x (state-half sigma in {0,1}).
Each core runs the full per-image pipeline for its batch element but only 8
of the 16 SSM states; partial y is AllReduce'd (per L-half) within core
pairs; the post-stack is computed redundantly on both cores of a pair.

Structure:
- everything is pipelined by L-halves front-to-back: image DMAs, the front
  conv (3 partition-paired PE taps over host-pre-shifted images, BN+ReLU in
  ACT, squares on DVE), packed LayerNorm stats (one paired PE matmul per
  chunk into [16,512] PSUM rows), rstd/-mu*rstd DRAM broadcasts, hn, the
  conv1d-fused xc taps (2 partition-paired PE matmuls over a duplicated
  LN output), the collapsed dt matmul + Exp/Ln softplus, and the B_j/C_j
  row broadcasts
- states 0-2 (first half) are emitted between the two phase halves so the
  DVE scan pipeline starts while the second half is still projecting
- per-state multiplies read bf16 SBUF broadcasts (DVE 2x mode); y =
  sum_j H_j*C_j accumulates into a full-PSUM [128,4096] f32 tile via
  identity matmuls
- z projection runs in the tail (PSUM is free then) overlapped with the
  per-half AllReduce + post + fused out_proj/residual matmuls

Self-contained: hardcodes all shapes; no sibling imports.
"""
import numpy as np

C = 64
DI = 128
DS = 16
DR = 4
B = 4
H = 64
W = 64
L = H * W
NS = 8            # states per core
NCORES = 8
LH = L // 2
IW = 64 + L + 64  # padded image width
EPS = 1e-5

_cached = {}


def _build_program(sim=False):
    import concourse.bass as bass
    import concourse.bacc as bacc
    import concourse.mybir as mybir
    import concourse.tile as tile

    dt = mybir.dt
    f32 = dt.float32
    bf16 = dt.bfloat16
    Act = mybir.ActivationFunctionType
    Alu = mybir.AluOpType

    nc = bacc.Bacc(None, target_bir_lowering=False)

    ximgs_d = nc.dram_tensor("ximgs", [128, 3 * IW], bf16,
                             kind="ExternalInput")
    cf32_d = nc.dram_tensor("cf32", [128, 24], f32, kind="ExternalInput")
    cbf_d = nc.dram_tensor("cbf", [128, 1040], bf16, kind="ExternalInput")
    out_d = nc.dram_tensor("out_f", [C, L], f32, kind="ExternalOutput")

    groups = [[0, 1], [2, 3], [4, 5], [6, 7]]

    with tile.TileContext(nc) as tc:
        with (
            tc.tile_pool(name="dram", bufs=1, space="DRAM") as dpool,
            tc.tile_pool(name="const", bufs=1) as cpool,
            tc.tile_pool(name="big", bufs=1) as bpool,
            tc.tile_pool(name="sm", bufs=1) as spool,
        ):
            bc_d = dpool.tile([16, L], bf16, tag="bc_scr")
            st_d = dpool.tile([8, 1024], bf16, tag="st_scr")
            yin_d = [dpool.tile([DI, LH], bf16, name=f"y_in{k}",
                                tag=f"y_in{k}") for k in range(2)]
            yout_d = [dpool.tile([DI, LH], bf16, name=f"y_out{k}",
                                 tag=f"y_out{k}") for k in range(2)]

            cf = cpool.tile([128, 24], f32)
            cb = cpool.tile([128, 1040], bf16)
            nc.sync.dma_start(cf[:], cf32_d[:])
            nc.sync.dma_start(cb[:], cbf_d[:])
            bn_s = cf[:, 0:1]
            bn_b = cf[:, 1:2]
            z_b = cf[:, 2:3]
            cd_b = cf[:, 3:4]
            dt_b = cf[:, 4:5]
            a_sc = cf[:, 5:13]
            Dp = cf[:, 13:14]
            ident = cb[:, 0:128]
            cwp = cb[:, 128:320]           # 3 paired front taps [128, 3*64]
            ipz = cb[0:C, 320:448]         # [64, 128] z projection
            tapp = cb[:, 448:704]          # 2 paired conv1d taps [128, 2*128]
            mdt = cb[:, 704:832]           # [128, 128] dt lhsT
            bcw = cb[:, 832:848]           # [128, 16] B/C rows lhsT
            opw = cb[:, 848:912]           # [128, 64] out_proj lhsT
            hotq = cb[:, 912:976]          # 8 x ([128,4] mu + [128,4] sq)

            warm = cpool.tile([128, 1], f32, tag="warm")
            nc.scalar.activation(warm[:], cf[:, 0:1], Act.Copy)
            warm2 = cpool.tile([128, 1], bf16, tag="warm2")
            nc.scalar.activation(warm2[:], cb[:, 0:1], Act.Copy)

            # ---- persistent activations ----
            SEQSQ = bpool.tile([128, L], bf16, name="SEQSQ", tag="SEQSQ")
            SEQ = SEQSQ[0:C, :]
            SQ = SEQSQ[C:128, :]
            HNP = bpool.tile([128, 12 + L], bf16, name="HNP", tag="HNP")
            ZS = bpool.tile([DI, L], bf16, name="ZS", tag="ZSp")
            XC = bpool.tile([DI, L], bf16, name="XC", tag="XCp")
            DT = bpool.tile([DI, L], bf16, name="DT", tag="DTp")
            U = bpool.tile([DI, L], bf16, name="U", tag="Up")
            XCD = bpool.tile([DI, L], bf16, name="XCD", tag="xcd")

            # rotating full tiles whose halves serve as per-half slots;
            # tag-aliased with early-phase dead tiles
            P1 = bpool.tile([128, IW], bf16, name="P1", tag="br0")
            P2 = bpool.tile([128, IW], bf16, name="P2", tag="br1")
            P3 = bpool.tile([128, IW], bf16, name="P3", tag="br2")
            RSTB = bpool.tile([C, L], bf16, name="RSTB", tag="cr0")
            MRSB = bpool.tile([C, L], bf16, name="MRSB", tag="cr1")
            BCSB = bpool.tile([16, L], bf16, name="BCSB", tag="cr2")
            ESB = bpool.tile([DI, 2048], f32, name="ESB", tag="tmp0")

            BRF = [bpool.tile([DI, L], bf16, name=f"BR{k}", tag=f"br{k}")
                   for k in range(3)]
            CRF = [bpool.tile([DI, L], bf16, name=f"CR{k}", tag=f"cr{k}")
                   for k in range(3)]
            Hs = [bpool.tile([DI, L], bf16, name=f"HH{k}", tag=f"h{k}")
                  for k in range(3)]
            TMPs = [bpool.tile([DI, L], bf16, name=f"TMP{k}", tag=f"tmp{k}")
                    for k in range(2)]
            DAs = [bpool.tile([DI, 2048], f32, name=f"DA{k}", tag=f"da{k}")
                   for k in range(4)]
            DBXs = [bpool.tile([DI, 2048], bf16, name=f"DBX{k}",
                               tag=f"dbx{k}") for k in range(4)]
            YSUM = [bpool.tile([DI, 2048], bf16, name=f"YSUM{k}",
                               tag=f"da{k}") for k in range(2)]
            YS = [bpool.tile([DI, 2048], bf16, name=f"YS{k}", tag=f"dbx{k}")
                  for k in range(2)]
            YSB = [bpool.tile([DI, 2048], bf16, name=f"YSB{k}",
                              tag=["tmp1", "da2"][k]) for k in range(2)]
            OUTC = [bpool.tile([C, 1024], f32, name="OUTC0", tag="dbx2"),
                    bpool.tile([C, 1024], f32, name="OUTC1", tag="dbx3")]

            def br_slot(j, hf):
                m = (2 * j + hf) % 6
                return BRF[m // 2][:, (m % 2) * LH:(m % 2 + 1) * LH]

            def cr_slot(j, hf):
                m = (2 * j + hf) % 6
                return CRF[m // 2][:, (m % 2) * LH:(m % 2 + 1) * LH]

            def bcast(dst, row, hf):
                src = bc_d[row:row + 1, hf * LH:(hf + 1) * LH]
                nc.gpsimd.dma_start(dst[:], src.to_broadcast((DI, LH)))

            nc.gpsimd.memset(HNP[:, 0:12], 0.0)

            # image DMAs, first-half columns first
            IWH = 64 + LH + 64
            for hf in range(2):
                c0 = IWH if hf else 0
                c1 = IW if hf else IWH
                for t, P in enumerate([P1, P2, P3]):
                    nc.sync.dma_start(P[:, c0:c1],
                                      ximgs_d[:, t * IW + c0:t * IW + c1])

            with (
                tc.tile_pool(name="psA", bufs=3, space="PSUM") as psA,
                tc.tile_pool(name="psSt", bufs=1, space="PSUM") as psSt,
                tc.tile_pool(name="psB", bufs=1, space="PSUM") as psB,
            ):
                st_ps = [psSt.tile([4, 1024], f32, name=f"st{h}",
                                   tag=f"st{h}") for h in range(2)]

                def front_half(hf):
                    for gg in range(4):
                        ch = hf * 4 + gg
                        pc = psA.tile([C, 512], f32, tag="mm")
                        sl0 = ch * 512
                        for tp in range(3):
                            nc.tensor.matmul(
                                pc[:], cwp[:, tp * C:(tp + 1) * C],
                                [P1, P2, P3][tp][:, 64 + sl0:64 + sl0 + 512],
                                start=(tp == 0), stop=(tp == 2))
                        gsl = slice(ch * 512, (ch + 1) * 512)
                        nc.scalar.activation(SEQ[:, gsl], pc[:], Act.Relu,
                                             bias=bn_b[0:C], scale=bn_s[0:C])
                        nc.vector.tensor_mul(SQ[:, gsl], SEQ[:, gsl],
                                             SEQ[:, gsl])
                        # token sums -> cols 0-511, sq sums -> cols 512-1023,
                        # row = ch%4 (keeps all reads partition-0-aligned)
                        nc.tensor.matmul(st_ps[hf][:, 0:512],
                                         hotq[:, ch * 8:ch * 8 + 4],
                                         SEQSQ[:, gsl],
                                         start=(gg == 0), stop=(gg == 3),
                                         skip_group_check=True)
                        nc.tensor.matmul(st_ps[hf][:, 512:1024],
                                         hotq[:, ch * 8 + 4:ch * 8 + 8],
                                         SEQSQ[:, gsl],
                                         start=(gg == 0), stop=(gg == 3),
                                         skip_group_check=True)
                    # LN smalls for this half ([4, 512], partition offset 0)
                    MU = spool.tile([4, 512], f32, name=f"MU{hf}", tag="MU")
                    MSQ = spool.tile([4, 512], f32, name=f"MSQ{hf}",
                                     tag="MSQ")
                    MU2 = spool.tile([4, 512], f32, name=f"MU2{hf}",
                                     tag="MU2")
                    PK = spool.tile([4, 1024], bf16, name=f"PK{hf}",
                                    tag="PK")
                    nc.vector.tensor_scalar_mul(MU[:], st_ps[hf][:, 0:512],
                                                1.0 / C)
                    nc.vector.tensor_scalar(MSQ[:], st_ps[hf][:, 512:1024],
                                            1.0 / C, EPS,
                                            op0=Alu.mult, op1=Alu.add)
                    nc.vector.tensor_mul(MU2[:], MU[:], MU[:])
                    nc.vector.tensor_tensor(MSQ[:], MSQ[:], MU2[:],
                                            op=Alu.subtract)
                    nc.scalar.activation(MU2[:], MSQ[:], Act.Sqrt)
                    with nc.allow_low_precision(reason="bf16 rstd bcast"):
                        nc.vector.reciprocal(PK[:, 0:512], MU2[:])
                    nc.vector.tensor_tensor(PK[:, 512:1024], MU[:],
                                            PK[:, 0:512], op=Alu.mult)
                    rsl = slice(hf * 4, hf * 4 + 4)
                    nc.sync.dma_start(st_d[rsl, :], PK[:])
                    hsl = slice(hf * LH, (hf + 1) * LH)
                    nc.gpsimd.dma_start(
                        RSTB[:, hsl].rearrange("p (a b) -> p a b", a=4),
                        st_d[rsl, 0:512].rearrange("a (c b) -> c a b", c=1)
                        .to_broadcast((C, 4, 512)))
                    nc.gpsimd.dma_start(
                        MRSB[:, hsl].rearrange("p (a b) -> p a b", a=4),
                        st_d[rsl, 512:1024].rearrange("a (c b) -> c a b", c=1)
                        .to_broadcast((C, 4, 512)))
                    psl = slice(8 + hf * LH, 8 + (hf + 1) * LH)
                    nc.vector.tensor_tensor(HNP[0:C, psl], SEQ[:, hsl],
                                            RSTB[:, hsl], op=Alu.mult)
                    nc.vector.tensor_tensor(HNP[0:C, psl], HNP[0:C, psl],
                                            MRSB[:, hsl], op=Alu.subtract)
                    # duplicated upper copy, shifted one column left
                    nc.sync.dma_start(HNP[C:128, 7 + hf * LH:7 + (hf + 1) * LH],
                                      HNP[0:C, 8 + hf * LH:8 + (hf + 1) * LH])

                def proj_half(hf):
                    hsl = slice(hf * LH, (hf + 1) * LH)
                    for gg in range(4):
                        ch = hf * 4 + gg
                        xp = psA.tile([DI, 512], f32, tag="mm")
                        b0 = 8 + ch * 512
                        nc.tensor.matmul(xp[:], tapp[:, 0:DI],
                                         HNP[:, b0 - 3:b0 - 3 + 512],
                                         start=True, stop=False)
                        nc.tensor.matmul(xp[:], tapp[:, DI:2 * DI],
                                         HNP[:, b0 - 1:b0 - 1 + 512],
                                         start=False, stop=True)
                        nc.scalar.activation(XC[:, ch * 512:(ch + 1) * 512],
                                             xp[:], Act.Silu, bias=cd_b)
                    for ch in range(hf * 4, hf * 4 + 4):
                        bp = psB.tile([16, 512], f32, tag="bc")
                        sl = slice(ch * 512, (ch + 1) * 512)
                        nc.tensor.matmul(bp[:], bcw, XC[:, sl],
                                         start=True, stop=True)
                        nc.vector.tensor_copy(BCSB[:, sl], bp[:])
                    nc.sync.dma_start(bc_d[:, hsl], BCSB[:, hsl])
                    for gg in range(4):
                        ch = hf * 4 + gg
                        dp = psA.tile([DI, 512], f32, tag="mm")
                        sl = slice(ch * 512, (ch + 1) * 512)
                        nc.tensor.matmul(dp[:], mdt, XC[:, sl],
                                         start=True, stop=True)
                        esl = ESB[:, gg * 512:(gg + 1) * 512]
                        nc.scalar.activation(esl, dp[:], Act.Exp, bias=dt_b)
                        if gg % 2 == 1:
                            dsl = slice((ch - 1) * 512, (ch + 1) * 512)
                            nc.scalar.activation(
                                DT[:, dsl], ESB[:, (gg - 1) * 512:
                                                 (gg + 1) * 512],
                                Act.Ln, bias=1.0)
                    nc.vector.tensor_tensor(U[:, hsl], DT[:, hsl], XC[:, hsl],
                                            op=Alu.mult)
                    nc.vector.tensor_scalar_mul(XCD[:, hsl], XC[:, hsl], Dp)

                def state_half(j, hf, slot=None):
                    sl = slice(hf * LH, (hf + 1) * LH)
                    hh = Hs[j % 3]
                    if slot is None:
                        slot = (2 * j + hf) % 4
                    da = DAs[slot]
                    nc.scalar.activation(da[:], DT[:, sl], Act.Exp,
                                         scale=a_sc[:, j:j + 1])
                    dbx = DBXs[slot]
                    nc.vector.tensor_tensor(dbx[:], U[:, sl],
                                            br_slot(j, hf), op=Alu.mult)
                    init = 0.0 if hf == 0 else hh[:, LH - 1:LH]
                    nc.vector.tensor_tensor_scan(hh[:, sl], da[:], dbx[:],
                                                 init, op0=Alu.mult,
                                                 op1=Alu.add)

                front_half(0)
                front_half(1)
                proj_half(0)
                bcast(br_slot(0, 0), 0, 0)
                bcast(cr_slot(0, 0), 8, 0)
                bcast(br_slot(1, 0), 1, 0)
                bcast(cr_slot(1, 0), 9, 0)
                bcast(br_slot(2, 0), 2, 0)
                state_half(0, 0)
                state_half(1, 0)
                state_half(2, 0, slot=3)
                proj_half(1)
                bcast(br_slot(0, 1), 0, 1)
                bcast(cr_slot(0, 1), 8, 1)

            # ================= state loop =================================
            with tc.tile_pool(name="psY", bufs=1, space="PSUM") as psY:
                y_ps = psY.tile([DI, L], f32, tag="y")
                for j in range(NS):
                    hh = Hs[j % 3]
                    tmp = TMPs[j % 2]
                    if j + 1 < NS:
                        bcast(br_slot(j + 1, 1), j + 1, 1)
                        bcast(cr_slot(j + 1, 1), 8 + j + 1, 1)
                    if j + 2 < NS:
                        bcast(cr_slot(j + 2, 0), 8 + j + 2, 0)
                    if j + 3 < NS:
                        bcast(br_slot(j + 3, 0), j + 3, 0)
                    if j >= 3:
                        state_half(j, 0)
                    state_half(j, 1)
                    for hf in range(2):
                        sl = slice(hf * LH, (hf + 1) * LH)
                        nc.vector.tensor_tensor(tmp[:, sl], hh[:, sl],
                                                cr_slot(j, hf), op=Alu.mult)
                        for ch in range(4):
                            psl = slice(hf * LH + ch * 512,
                                        hf * LH + (ch + 1) * 512)
                            nc.tensor.matmul(y_ps[:, psl], ident,
                                             tmp[:, psl], start=(j == 0),
                                             stop=(j == NS - 1),
                                             skip_group_check=True)
                for hf in range(2):
                    sl = slice(hf * LH, (hf + 1) * LH)
                    nc.scalar.activation(YSB[hf][:], y_ps[:, sl], Act.Copy)
                    nc.sync.dma_start(yin_d[hf][:], YSB[hf][:])
                    if sim:
                        nc.sync.dma_start(yout_d[hf][:], yin_d[hf][:])
                    else:
                        nc.gpsimd.collective_compute(
                            "AllReduce", Alu.add, replica_groups=groups,
                            ins=[yin_d[hf].opt()],
                            outs=[yout_d[hf].opt()])

            # ====== tail: z projection + post + out, per half =============
            with (
                tc.tile_pool(name="psC", bufs=3, space="PSUM") as psC,
                tc.tile_pool(name="psZ", bufs=2, space="PSUM") as psZ,
            ):
                for hf in range(2):
                    sl = slice(hf * LH, (hf + 1) * LH)
                    for gg in range(4):
                        ch = hf * 4 + gg
                        zp = psZ.tile([DI, 512], f32, tag="zmm")
                        b0 = 8 + ch * 512
                        nc.tensor.matmul(zp[:], ipz, HNP[0:C, b0:b0 + 512],
                                         start=True, stop=True)
                        nc.scalar.activation(ZS[:, ch * 512:(ch + 1) * 512],
                                             zp[:], Act.Silu, bias=z_b)
                    # keep the PE clock ramped through the collective gap so
                    # the out_proj matmuls run at full p-state
                    for k in range(8 if hf == 0 else 2):
                        wp = psZ.tile([DI, 512], f32, name=f"w{hf}{k}",
                                      tag="zmm")
                        nc.tensor.matmul(wp[:], ident, U[:, 0:512],
                                         start=True, stop=True)
                    nc.sync.dma_start(YSUM[hf][:], yout_d[hf][:])
                    nc.vector.tensor_add(YS[hf][:], YSUM[hf][:], XCD[:, sl])
                    nc.vector.tensor_mul(YS[hf][:], YS[hf][:], ZS[:, sl])
                    for gg in range(2):
                        op_ps = psC.tile([C, 1024], f32, tag="op")
                        for s in range(2):
                            c0 = gg * 1024 + s * 512
                            osl = op_ps[:, s * 512:(s + 1) * 512]
                            nc.tensor.matmul(osl, opw,
                                             YS[hf][:, c0:c0 + 512],
                                             start=True, stop=False,
                                             skip_group_check=True)
                            nc.tensor.matmul(osl, ident[0:C, 0:C],
                                             SEQ[:, hf * LH + c0:
                                                 hf * LH + c0 + 512],
                                             start=False, stop=True,
                                             skip_group_check=True)
                        oc = OUTC[gg]
                        nc.scalar.activation(oc[:], op_ps[:], Act.Copy)
                        nc.sync.dma_start(
                            out_d[:, hf * LH + gg * 1024:
                                  hf * LH + (gg + 1) * 1024], oc[:])

    nc.compile()
    return nc


def _host_precompute(inp):
    import ml_dtypes
    f = lambda k: np.asarray(inp[k], np.float32)
    bf = lambda a: np.ascontiguousarray(a.astype(ml_dtypes.bfloat16))
    w1 = f("conv_w")[:, :, 0, 0]
    wh = f("dwh_w")[:, 0, :, 0]
    ww = f("dww_w")[:, 0, 0, :]
    taps = [
        w1 * (1.0 + wh[:, 1] + ww[:, 1])[None, :],   # center
        w1 * wh[:, 0][None, :],                       # up   (reads h-1)
        w1 * wh[:, 2][None, :],                       # down (reads h+1)
        w1 * ww[:, 0][None, :],                       # left
        w1 * ww[:, 2][None, :],                       # right
    ]
    z64 = np.zeros((C, C), np.float32)
    cwp = np.concatenate([
        np.concatenate([taps[0].T, taps[1].T], axis=0),
        np.concatenate([taps[2].T, taps[3].T], axis=0),
        np.concatenate([taps[4].T, z64], axis=0),
    ], axis=1)                                        # [128, 3*64]
    btot = f("conv_b") + w1 @ (f("dwh_b") + f("dww_b"))
    s_bn = f("bn_g") / np.sqrt(f("bn_v") + EPS)
    bn_bias = s_bn * (btot - f("bn_m")) + f("bn_b")
    ipw = f("in_proj_w")
    ipw_g = ipw * f("ln_g")[None, :]
    ip_bias = ipw @ f("ln_b")
    cdw = f("convd_w")[:, 0, :]                       # [128, 4]
    tk = [(cdw[:, k:k + 1] * ipw_g[:DI]).T for k in range(4)]  # [64,128]
    tapp = np.concatenate([
        np.concatenate([tk[0], tk[1]], axis=0),
        np.concatenate([tk[2], tk[3]], axis=0),
    ], axis=1)                                        # [128, 2*128]
    cd_eff = f("convd_b") + ip_bias[:DI] * cdw.sum(1)
    xpw = f("x_proj_w")
    mdt = (f("dt_proj_w") @ xpw[:DR]).T               # [128, 128]
    a_full = -np.exp(np.asarray(inp["A_log"], np.float32))
    # per chunk ch: [128,4] mu block (ones col ch%4, top rows) then [128,4]
    # sq block (ones col ch%4, bottom rows)
    hotq = np.zeros((128, 64), np.float32)
    for ch in range(8):
        hotq[0:C, ch * 8 + ch % 4] = 1.0
        hotq[C:128, ch * 8 + 4 + ch % 4] = 1.0

    per_sigma = []
    for sg in range(2):
        s_lo = sg * NS
        cf32 = np.zeros((128, 24), np.float32)
        cf32[:C, 0] = s_bn
        cf32[:C, 1] = bn_bias
        cf32[:, 2] = ip_bias[DI:]
        cf32[:, 3] = cd_eff
        cf32[:, 4] = f("dt_proj_b")
        for j in range(NS):
            cf32[:, 5 + j] = a_full[:, s_lo + j]
        cf32[:, 13] = f("Dp")

        cbf = np.zeros((128, 1040), np.float32)
        cbf[:, 0:128] = np.eye(128, dtype=np.float32)
        cbf[:, 128:320] = cwp
        cbf[:C, 320:448] = ipw_g[DI:].T
        cbf[:, 448:704] = tapp
        cbf[:, 704:832] = mdt
        bc_rows = np.concatenate([xpw[DR + s_lo:DR + s_lo + NS],
                                  xpw[DR + DS + s_lo:DR + DS + s_lo + NS]],
                                 axis=0)
        cbf[:, 832:848] = bc_rows.T
        cbf[:, 848:912] = f("out_proj_w").T
        cbf[:, 912:976] = hotq
        per_sigma.append(dict(cf32=cf32, cbf=bf(cbf)))
    return per_sigma


def _pack_images(xb):
    """3 paired tiles [128, IW]: reading cols 64+t..64+t+511 yields
    (ctr[t], ctr[t-64]), (ctr[t+64], lf[t]), (rt[t], 0)."""
    import ml_dtypes
    ctr = xb.reshape(C, L)
    lf = np.zeros((C, H, W), np.float32)
    lf[:, :, 1:] = xb[:, :, :-1]
    rt = np.zeros((C, H, W), np.float32)
    rt[:, :, :-1] = xb[:, :, 1:]
    out = np.zeros((128, 3 * IW), np.float32)
    out[0:C, 64:64 + L] = ctr
    out[C:128, 128:128 + L] = ctr
    out[0:C, IW + 0:IW + L] = ctr
    out[C:128, IW + 64:IW + 64 + L] = lf.reshape(C, L)
    out[0:C, 2 * IW + 64:2 * IW + 64 + L] = rt.reshape(C, L)
    return np.ascontiguousarray(out.astype(ml_dtypes.bfloat16))


TRACE = False
LAST_EXEC_NS = None
LAST_TRACE_DIR = None


def kernel(**inputs):
    global LAST_EXEC_NS, LAST_TRACE_DIR
    from concourse.bass_utils import run_bass_kernel_spmd

    if "nc" not in _cached:
        _cached["nc"] = _build_program()
    nc = _cached["nc"]

    per_sigma = _host_precompute(inputs)
    x = np.asarray(inputs["x"], np.float32)
    in_maps = []
    for c in range(NCORES):
        b, sg = c // 2, c % 2
        m = dict(per_sigma[sg])
        m["ximgs"] = _pack_images(x[b])
        in_maps.append(m)

    kw = {}
    if TRACE:
        import tempfile
        LAST_TRACE_DIR = tempfile.mkdtemp(prefix="bass_trace_")
        kw = dict(trace=True, tmpdir=LAST_TRACE_DIR)
    r = run_bass_kernel_spmd(nc, in_maps, list(range(NCORES)), **kw)
    if r.exec_time_ns is not None:
        LAST_EXEC_NS = r.exec_time_ns
    res = r.results
    out = np.empty((B, C, H, W), np.float32)
    for b in range(B):
        out[b] = np.asarray(res[2 * b]["out_f"], np.float32).reshape(C, H, W)
    return out


# revision 50
# speedup vs baseline: 1.5840x; 1.0022x over previous
"""Trainium2 Bass kernel for nn_DecoderBlock_Mamba (AxialDW conv + 1x1 conv +
BN + ReLU + LN + Mamba selective scan + residual).

Sharding: 8 cores = (batch b in 0..3) x (state-half sigma in {0,1}).
Each core runs the full per-image pipeline for its batch element but only 8
of the 16 SSM states; partial y is AllReduce'd (per L-half) within core
pairs; the post-stack is computed redundantly on both cores of a pair.

Structure:
- everything is pipelined by L-halves front-to-back: image DMAs, the front
  conv (3 partition-paired PE taps over host-pre-shifted images, BN+ReLU in
  ACT, squares on DVE), packed LayerNorm stats (one paired PE matmul per
  chunk into [16,512] PSUM rows), rstd/-mu*rstd DRAM broadcasts, hn, the
  conv1d-fused xc taps (2 partition-paired PE matmuls over a duplicated
  LN output), the collapsed dt matmul + Exp/Ln softplus, and the B_j/C_j
  row broadcasts
- states 0-2 (first half) are emitted between the two phase halves so the
  DVE scan pipeline starts while the second half is still projecting
- per-state multiplies read bf16 SBUF broadcasts (DVE 2x mode); y =
  sum_j H_j*C_j accumulates into a full-PSUM [128,4096] f32 tile via
  identity matmuls
- z projection runs in the tail (PSUM is free then) overlapped with the
  per-half AllReduce + post + fused out_proj/residual matmuls

Self-contained: hardcodes all shapes; no sibling imports.
"""
import numpy as np

C = 64
DI = 128
DS = 16
DR = 4
B = 4
H = 64
W = 64
L = H * W
NS = 8            # states per core
NCORES = 8
LH = L // 2
IW = 64 + L + 64  # padded image width
EPS = 1e-5

_cached = {}


def _build_program(sim=False):
    import concourse.bass as bass
    import concourse.bacc as bacc
    import concourse.mybir as mybir
    import concourse.tile as tile

    dt = mybir.dt
    f32 = dt.float32
    bf16 = dt.bfloat16
    Act = mybir.ActivationFunctionType
    Alu = mybir.AluOpType

    nc = bacc.Bacc(None, target_bir_lowering=False)

    ximgs_d = nc.dram_tensor("ximgs", [128, 3 * IW], bf16,
                             kind="ExternalInput")
    cf32_d = nc.dram_tensor("cf32", [128, 24], f32, kind="ExternalInput")
    cbf_d = nc.dram_tensor("cbf", [128, 1040], bf16, kind="ExternalInput")
    out_d = nc.dram_tensor("out_f", [C, L], f32, kind="ExternalOutput")

    groups = [[0, 1], [2, 3], [4, 5], [6, 7]]

    with tile.TileContext(nc) as tc:
        with (
            tc.tile_pool(name="dram", bufs=1, space="DRAM") as dpool,
            tc.tile_pool(name="const", bufs=1) as cpool,
            tc.tile_pool(name="big", bufs=1) as bpool,
            tc.tile_pool(name="sm", bufs=1) as spool,
        ):
            bc_d = dpool.tile([16, L], bf16, tag="bc_scr")
            st_d = dpool.tile([8, 1024], bf16, tag="st_scr")
            yin_d = [dpool.tile([DI, LH], bf16, name=f"y_in{k}",
                                tag=f"y_in{k}") for k in range(2)]
            yout_d = [dpool.tile([DI, LH], bf16, name=f"y_out{k}",
                                 tag=f"y_out{k}") for k in range(2)]

            cf = cpool.tile([128, 24], f32)
            cb = cpool.tile([128, 1040], bf16)
            nc.sync.dma_start(cf[:], cf32_d[:])
            nc.sync.dma_start(cb[:], cbf_d[:])
            bn_s = cf[:, 0:1]
            bn_b = cf[:, 1:2]
            z_b = cf[:, 2:3]
            cd_b = cf[:, 3:4]
            dt_b = cf[:, 4:5]
            a_sc = cf[:, 5:13]
            Dp = cf[:, 13:14]
            ident = cb[:, 0:128]
            cwp = cb[:, 128:320]           # 3 paired front taps [128, 3*64]
            ipz = cb[0:C, 320:448]         # [64, 128] z projection
            tapp = cb[:, 448:704]          # 2 paired conv1d taps [128, 2*128]
            mdt = cb[:, 704:832]           # [128, 128] dt lhsT
            bcw = cb[:, 832:848]           # [128, 16] B/C rows lhsT
            opw = cb[:, 848:912]           # [128, 64] out_proj lhsT
            hotq = cb[:, 912:976]          # 8 x ([128,4] mu + [128,4] sq)

            warm = cpool.tile([128, 1], f32, tag="warm")
            nc.scalar.activation(warm[:], cf[:, 0:1], Act.Copy)
            warm2 = cpool.tile([128, 1], bf16, tag="warm2")
            nc.scalar.activation(warm2[:], cb[:, 0:1], Act.Copy)

            # ---- persistent activations ----
            SEQSQ = bpool.tile([128, L], bf16, name="SEQSQ", tag="SEQSQ")
            SEQ = SEQSQ[0:C, :]
            SQ = SEQSQ[C:128, :]
            HNP = bpool.tile([128, 12 + L], bf16, name="HNP", tag="HNP")
            ZS = bpool.tile([DI, L], bf16, name="ZS", tag="ZSp")
            XC = bpool.tile([DI, L], bf16, name="XC", tag="XCp")
            DT = bpool.tile([DI, L], bf16, name="DT", tag="DTp")
            U = bpool.tile([DI, L], bf16, name="U", tag="Up")
            XCD = bpool.tile([DI, L], bf16, name="XCD", tag="xcd")

            # rotating full tiles whose halves serve as per-half slots;
            # tag-aliased with early-phase dead tiles
            P1 = bpool.tile([128, IW], bf16, name="P1", tag="br0")
            P2 = bpool.tile([128, IW], bf16, name="P2", tag="br1")
            P3 = bpool.tile([128, IW], bf16, name="P3", tag="br2")
            RSTB = bpool.tile([C, L], bf16, name="RSTB", tag="cr0")
            MRSB = bpool.tile([C, L], bf16, name="MRSB", tag="cr1")
            BCSB = bpool.tile([16, L], bf16, name="BCSB", tag="cr2")
            ESB = bpool.tile([DI, 2048], f32, name="ESB", tag="tmp0")

            BRF = [bpool.tile([DI, L], bf16, name=f"BR{k}", tag=f"br{k}")
                   for k in range(3)]
            CRF = [bpool.tile([DI, L], bf16, name=f"CR{k}", tag=f"cr{k}")
                   for k in range(3)]
            Hs = [bpool.tile([DI, L], bf16, name=f"HH{k}", tag=f"h{k}")
                  for k in range(3)]
            TMPs = [bpool.tile([DI, L], bf16, name=f"TMP{k}", tag=f"tmp{k}")
                    for k in range(2)]
            DAs = [bpool.tile([DI, 2048], f32, name=f"DA{k}", tag=f"da{k}")
                   for k in range(4)]
            DBXs = [bpool.tile([DI, 2048], bf16, name=f"DBX{k}",
                               tag=f"dbx{k}") for k in range(4)]
            YSUM = [bpool.tile([DI, 2048], bf16, name=f"YSUM{k}",
                               tag=f"da{k}") for k in range(2)]
            YS = [bpool.tile([DI, 2048], bf16, name=f"YS{k}", tag=f"dbx{k}")
                  for k in range(2)]
            YSB = [bpool.tile([DI, 2048], bf16, name=f"YSB{k}",
                              tag=["tmp1", "da2"][k]) for k in range(2)]
            OUTC = [bpool.tile([C, 1024], f32, name="OUTC0", tag="dbx2"),
                    bpool.tile([C, 1024], f32, name="OUTC1", tag="dbx3")]

            def br_slot(j, hf):
                m = (2 * j + hf) % 6
                return BRF[m // 2][:, (m % 2) * LH:(m % 2 + 1) * LH]

            def cr_slot(j, hf):
                m = (2 * j + hf) % 6
                return CRF[m // 2][:, (m % 2) * LH:(m % 2 + 1) * LH]

            def bcast(dst, row, hf):
                src = bc_d[row:row + 1, hf * LH:(hf + 1) * LH]
                nc.gpsimd.dma_start(dst[:], src.to_broadcast((DI, LH)))

            nc.gpsimd.memset(HNP[:, 0:12], 0.0)

            # image DMAs, first-half columns first
            IWH = 64 + LH + 64
            for hf in range(2):
                c0 = IWH if hf else 0
                c1 = IW if hf else IWH
                for t, P in enumerate([P1, P2, P3]):
                    nc.sync.dma_start(P[:, c0:c1],
                                      ximgs_d[:, t * IW + c0:t * IW + c1])

            with (
                tc.tile_pool(name="psA", bufs=3, space="PSUM") as psA,
                tc.tile_pool(name="psSt", bufs=1, space="PSUM") as psSt,
                tc.tile_pool(name="psB", bufs=1, space="PSUM") as psB,
            ):
                st_ps = [psSt.tile([4, 1024], f32, name=f"st{h}",
                                   tag=f"st{h}") for h in range(2)]


                def front_half(hf):
                    for gg in range(4):
                        ch = hf * 4 + gg
                        pc = psA.tile([C, 512], f32, tag="mm")
                        sl0 = ch * 512
                        for tp in range(3):
                            nc.tensor.matmul(
                                pc[:], cwp[:, tp * C:(tp + 1) * C],
                                [P1, P2, P3][tp][:, 64 + sl0:64 + sl0 + 512],
                                start=(tp == 0), stop=(tp == 2))
                        gsl = slice(ch * 512, (ch + 1) * 512)
                        nc.scalar.activation(SEQ[:, gsl], pc[:], Act.Relu,
                                             bias=bn_b[0:C], scale=bn_s[0:C])
                        nc.vector.tensor_mul(SQ[:, gsl], SEQ[:, gsl],
                                             SEQ[:, gsl])
                        # token sums -> cols 0-511, sq sums -> cols 512-1023,
                        # row = ch%4 (keeps all reads partition-0-aligned)
                        nc.tensor.matmul(st_ps[hf][:, 0:512],
                                         hotq[:, ch * 8:ch * 8 + 4],
                                         SEQSQ[:, gsl],
                                         start=(gg == 0), stop=(gg == 3),
                                         skip_group_check=True)
                        nc.tensor.matmul(st_ps[hf][:, 512:1024],
                                         hotq[:, ch * 8 + 4:ch * 8 + 8],
                                         SEQSQ[:, gsl],
                                         start=(gg == 0), stop=(gg == 3),
                                         skip_group_check=True)
                    # LN smalls for this half ([4, 512], partition offset 0)
                    MU = spool.tile([4, 512], f32, name=f"MU{hf}", tag="MU")
                    MSQ = spool.tile([4, 512], f32, name=f"MSQ{hf}",
                                     tag="MSQ")
                    MU2 = spool.tile([4, 512], f32, name=f"MU2{hf}",
                                     tag="MU2")
                    PK = spool.tile([4, 1024], bf16, name=f"PK{hf}",
                                    tag="PK")
                    nc.vector.tensor_scalar_mul(MU[:], st_ps[hf][:, 0:512],
                                                1.0 / C)
                    nc.vector.tensor_scalar(MSQ[:], st_ps[hf][:, 512:1024],
                                            1.0 / C, EPS,
                                            op0=Alu.mult, op1=Alu.add)
                    nc.vector.tensor_mul(MU2[:], MU[:], MU[:])
                    nc.vector.tensor_tensor(MSQ[:], MSQ[:], MU2[:],
                                            op=Alu.subtract)
                    nc.scalar.activation(MU2[:], MSQ[:], Act.Sqrt)
                    with nc.allow_low_precision(reason="bf16 rstd bcast"):
                        nc.vector.reciprocal(PK[:, 0:512], MU2[:])
                    nc.vector.tensor_tensor(PK[:, 512:1024], MU[:],
                                            PK[:, 0:512], op=Alu.mult)
                    rsl = slice(hf * 4, hf * 4 + 4)
                    nc.sync.dma_start(st_d[rsl, :], PK[:])
                    hsl = slice(hf * LH, (hf + 1) * LH)
                    nc.gpsimd.dma_start(
                        RSTB[:, hsl].rearrange("p (a b) -> p a b", a=4),
                        st_d[rsl, 0:512].rearrange("a (c b) -> c a b", c=1)
                        .to_broadcast((C, 4, 512)))
                    nc.gpsimd.dma_start(
                        MRSB[:, hsl].rearrange("p (a b) -> p a b", a=4),
                        st_d[rsl, 512:1024].rearrange("a (c b) -> c a b", c=1)
                        .to_broadcast((C, 4, 512)))
                    psl = slice(8 + hf * LH, 8 + (hf + 1) * LH)
                    nc.vector.tensor_tensor(HNP[0:C, psl], SEQ[:, hsl],
                                            RSTB[:, hsl], op=Alu.mult)
                    nc.vector.tensor_tensor(HNP[0:C, psl], HNP[0:C, psl],
                                            MRSB[:, hsl], op=Alu.subtract)
                    # duplicated upper copy, shifted one column left
                    nc.sync.dma_start(HNP[C:128, 7 + hf * LH:7 + (hf + 1) * LH],
                                      HNP[0:C, 8 + hf * LH:8 + (hf + 1) * LH])

                def proj_half(hf):
                    hsl = slice(hf * LH, (hf + 1) * LH)
                    for gg in range(4):
                        ch = hf * 4 + gg
                        xp = psA.tile([DI, 512], f32, tag="mm")
                        b0 = 8 + ch * 512
                        nc.tensor.matmul(xp[:], tapp[:, 0:DI],
                                         HNP[:, b0 - 3:b0 - 3 + 512],
                                         start=True, stop=False)
                        nc.tensor.matmul(xp[:], tapp[:, DI:2 * DI],
                                         HNP[:, b0 - 1:b0 - 1 + 512],
                                         start=False, stop=True)
                        nc.scalar.activation(XC[:, ch * 512:(ch + 1) * 512],
                                             xp[:], Act.Silu, bias=cd_b)
                    for ch in range(hf * 4, hf * 4 + 4):
                        bp = psB.tile([16, 512], f32, tag="bc")
                        sl = slice(ch * 512, (ch + 1) * 512)
                        nc.tensor.matmul(bp[:], bcw, XC[:, sl],
                                         start=True, stop=True)
                        nc.vector.tensor_copy(BCSB[:, sl], bp[:])
                    nc.sync.dma_start(bc_d[:, hsl], BCSB[:, hsl])
                    for gg in range(4):
                        ch = hf * 4 + gg
                        dp = psA.tile([DI, 512], f32, tag="mm")
                        sl = slice(ch * 512, (ch + 1) * 512)
                        nc.tensor.matmul(dp[:], mdt, XC[:, sl],
                                         start=True, stop=True)
                        esl = ESB[:, gg * 512:(gg + 1) * 512]
                        nc.scalar.activation(esl, dp[:], Act.Exp, bias=dt_b)
                        if gg % 2 == 1:
                            dsl = slice((ch - 1) * 512, (ch + 1) * 512)
                            nc.scalar.activation(
                                DT[:, dsl], ESB[:, (gg - 1) * 512:
                                                 (gg + 1) * 512],
                                Act.Ln, bias=1.0)
                    nc.vector.tensor_tensor(U[:, hsl], DT[:, hsl], XC[:, hsl],
                                            op=Alu.mult)
                    nc.vector.tensor_scalar_mul(XCD[:, hsl], XC[:, hsl], Dp)

                def state_half(j, hf, slot=None):
                    sl = slice(hf * LH, (hf + 1) * LH)
                    hh = Hs[j % 3]
                    if slot is None:
                        slot = (2 * j + hf) % 4
                    da = DAs[slot]
                    nc.scalar.activation(da[:], DT[:, sl], Act.Exp,
                                         scale=a_sc[:, j:j + 1])
                    dbx = DBXs[slot]
                    nc.vector.tensor_tensor(dbx[:], U[:, sl],
                                            br_slot(j, hf), op=Alu.mult)
                    init = 0.0 if hf == 0 else hh[:, LH - 1:LH]
                    nc.vector.tensor_tensor_scan(hh[:, sl], da[:], dbx[:],
                                                 init, op0=Alu.mult,
                                                 op1=Alu.add)

                front_half(0)
                front_half(1)
                proj_half(0)
                bcast(br_slot(0, 0), 0, 0)
                bcast(cr_slot(0, 0), 8, 0)
                bcast(br_slot(1, 0), 1, 0)
                bcast(cr_slot(1, 0), 9, 0)
                bcast(br_slot(2, 0), 2, 0)
                state_half(0, 0)
                state_half(1, 0)
                state_half(2, 0, slot=3)
                proj_half(1)
                bcast(br_slot(0, 1), 0, 1)
                bcast(cr_slot(0, 1), 8, 1)

            # ================= state loop =================================
            with tc.tile_pool(name="psY", bufs=1, space="PSUM") as psY:
                y_ps = psY.tile([DI, L], f32, tag="y")
                for j in range(NS):
                    hh = Hs[j % 3]
                    tmp = TMPs[j % 2]
                    if j + 1 < NS:
                        bcast(br_slot(j + 1, 1), j + 1, 1)
                        bcast(cr_slot(j + 1, 1), 8 + j + 1, 1)
                    if j + 2 < NS:
                        bcast(cr_slot(j + 2, 0), 8 + j + 2, 0)
                    if j + 3 < NS:
                        bcast(br_slot(j + 3, 0), j + 3, 0)
                    if j >= 3:
                        state_half(j, 0)
                    state_half(j, 1)
                    for hf in range(2):
                        sl = slice(hf * LH, (hf + 1) * LH)
                        nc.vector.tensor_tensor(tmp[:, sl], hh[:, sl],
                                                cr_slot(j, hf), op=Alu.mult)
                        for ch in range(4):
                            psl = slice(hf * LH + ch * 512,
                                        hf * LH + (ch + 1) * 512)
                            nc.tensor.matmul(y_ps[:, psl], ident,
                                             tmp[:, psl], start=(j == 0),
                                             stop=(j == NS - 1),
                                             skip_group_check=True)
                for hf in range(2):
                    sl = slice(hf * LH, (hf + 1) * LH)
                    nc.scalar.activation(YSB[hf][:], y_ps[:, sl], Act.Copy)
                    nc.sync.dma_start(yin_d[hf][:], YSB[hf][:])
                    if sim:
                        nc.sync.dma_start(yout_d[hf][:], yin_d[hf][:])
                    else:
                        nc.gpsimd.collective_compute(
                            "AllReduce", Alu.add, replica_groups=groups,
                            ins=[yin_d[hf].opt()],
                            outs=[yout_d[hf].opt()])

            # ====== tail: z projection + post + out, per half =============
            with (
                tc.tile_pool(name="psC", bufs=3, space="PSUM") as psC,
                tc.tile_pool(name="psZ", bufs=2, space="PSUM") as psZ,
            ):
                for hf in range(2):
                    sl = slice(hf * LH, (hf + 1) * LH)
                    for gg in range(4):
                        ch = hf * 4 + gg
                        zp = psZ.tile([DI, 512], f32, tag="zmm")
                        b0 = 8 + ch * 512
                        nc.tensor.matmul(zp[:], ipz, HNP[0:C, b0:b0 + 512],
                                         start=True, stop=True)
                        nc.scalar.activation(ZS[:, ch * 512:(ch + 1) * 512],
                                             zp[:], Act.Silu, bias=z_b)
                    # keep the PE clock ramped through the collective gap so
                    # the out_proj matmuls run at full p-state
                    for k in range(8 if hf == 0 else 2):
                        wp = psZ.tile([DI, 512], f32, name=f"w{hf}{k}",
                                      tag="zmm")
                        nc.tensor.matmul(wp[:], ident, U[:, 0:512],
                                         start=True, stop=True)
                    for q in range(2):
                        qsl = slice(q * 1024, (q + 1) * 1024)
                        nc.sync.dma_start(YSUM[hf][:, qsl],
                                          yout_d[hf][:, qsl])
                        nc.vector.tensor_add(YS[hf][:, qsl],
                                             YSUM[hf][:, qsl],
                                             XCD[:, hf * LH + q * 1024:
                                                 hf * LH + (q + 1) * 1024])
                        nc.vector.tensor_mul(YS[hf][:, qsl], YS[hf][:, qsl],
                                             ZS[:, hf * LH + q * 1024:
                                                hf * LH + (q + 1) * 1024])
                    for gg in range(2):
                        op_ps = psC.tile([C, 1024], f32, tag="op")
                        for s in range(2):
                            c0 = gg * 1024 + s * 512
                            osl = op_ps[:, s * 512:(s + 1) * 512]
                            nc.tensor.matmul(osl, opw,
                                             YS[hf][:, c0:c0 + 512],
                                             start=True, stop=False,
                                             skip_group_check=True)
                            nc.tensor.matmul(osl, ident[0:C, 0:C],
                                             SEQ[:, hf * LH + c0:
                                                 hf * LH + c0 + 512],
                                             start=False, stop=True,
                                             skip_group_check=True)
                        oc = OUTC[gg]
                        if gg == 0:
                            nc.scalar.activation(oc[:], op_ps[:], Act.Copy)
                        else:
                            nc.vector.tensor_copy(oc[:], op_ps[:])
                        nc.sync.dma_start(
                            out_d[:, hf * LH + gg * 1024:
                                  hf * LH + (gg + 1) * 1024], oc[:])

    nc.compile()
    return nc


def _host_precompute(inp):
    import ml_dtypes
    f = lambda k: np.asarray(inp[k], np.float32)
    bf = lambda a: np.ascontiguousarray(a.astype(ml_dtypes.bfloat16))
    w1 = f("conv_w")[:, :, 0, 0]
    wh = f("dwh_w")[:, 0, :, 0]
    ww = f("dww_w")[:, 0, 0, :]
    taps = [
        w1 * (1.0 + wh[:, 1] + ww[:, 1])[None, :],   # center
        w1 * wh[:, 0][None, :],                       # up   (reads h-1)
        w1 * wh[:, 2][None, :],                       # down (reads h+1)
        w1 * ww[:, 0][None, :],                       # left
        w1 * ww[:, 2][None, :],                       # right
    ]
    z64 = np.zeros((C, C), np.float32)
    cwp = np.concatenate([
        np.concatenate([taps[0].T, taps[1].T], axis=0),
        np.concatenate([taps[2].T, taps[3].T], axis=0),
        np.concatenate([taps[4].T, z64], axis=0),
    ], axis=1)                                        # [128, 3*64]
    btot = f("conv_b") + w1 @ (f("dwh_b") + f("dww_b"))
    s_bn = f("bn_g") / np.sqrt(f("bn_v") + EPS)
    bn_bias = s_bn * (btot - f("bn_m")) + f("bn_b")
    ipw = f("in_proj_w")
    ipw_g = ipw * f("ln_g")[None, :]
    ip_bias = ipw @ f("ln_b")
    cdw = f("convd_w")[:, 0, :]                       # [128, 4]
    tk = [(cdw[:, k:k + 1] * ipw_g[:DI]).T for k in range(4)]  # [64,128]
    tapp = np.concatenate([
        np.concatenate([tk[0], tk[1]], axis=0),
        np.concatenate([tk[2], tk[3]], axis=0),
    ], axis=1)                                        # [128, 2*128]
    cd_eff = f("convd_b") + ip_bias[:DI] * cdw.sum(1)
    xpw = f("x_proj_w")
    mdt = (f("dt_proj_w") @ xpw[:DR]).T               # [128, 128]
    a_full = -np.exp(np.asarray(inp["A_log"], np.float32))
    # per chunk ch: [128,4] mu block (ones col ch%4, top rows) then [128,4]
    # sq block (ones col ch%4, bottom rows)
    hotq = np.zeros((128, 64), np.float32)
    for ch in range(8):
        hotq[0:C, ch * 8 + ch % 4] = 1.0
        hotq[C:128, ch * 8 + 4 + ch % 4] = 1.0

    per_sigma = []
    for sg in range(2):
        s_lo = sg * NS
        cf32 = np.zeros((128, 24), np.float32)
        cf32[:C, 0] = s_bn
        cf32[:C, 1] = bn_bias
        cf32[:, 2] = ip_bias[DI:]
        cf32[:, 3] = cd_eff
        cf32[:, 4] = f("dt_proj_b")
        for j in range(NS):
            cf32[:, 5 + j] = a_full[:, s_lo + j]
        cf32[:, 13] = f("Dp")

        cbf = np.zeros((128, 1040), np.float32)
        cbf[:, 0:128] = np.eye(128, dtype=np.float32)
        cbf[:, 128:320] = cwp
        cbf[:C, 320:448] = ipw_g[DI:].T
        cbf[:, 448:704] = tapp
        cbf[:, 704:832] = mdt
        bc_rows = np.concatenate([xpw[DR + s_lo:DR + s_lo + NS],
                                  xpw[DR + DS + s_lo:DR + DS + s_lo + NS]],
                                 axis=0)
        cbf[:, 832:848] = bc_rows.T
        cbf[:, 848:912] = f("out_proj_w").T
        cbf[:, 912:976] = hotq
        per_sigma.append(dict(cf32=cf32, cbf=bf(cbf)))
    return per_sigma


def _pack_images(xb):
    """3 paired tiles [128, IW]: reading cols 64+t..64+t+511 yields
    (ctr[t], ctr[t-64]), (ctr[t+64], lf[t]), (rt[t], 0)."""
    import ml_dtypes
    ctr = xb.reshape(C, L)
    lf = np.zeros((C, H, W), np.float32)
    lf[:, :, 1:] = xb[:, :, :-1]
    rt = np.zeros((C, H, W), np.float32)
    rt[:, :, :-1] = xb[:, :, 1:]
    out = np.zeros((128, 3 * IW), np.float32)
    out[0:C, 64:64 + L] = ctr
    out[C:128, 128:128 + L] = ctr
    out[0:C, IW + 0:IW + L] = ctr
    out[C:128, IW + 64:IW + 64 + L] = lf.reshape(C, L)
    out[0:C, 2 * IW + 64:2 * IW + 64 + L] = rt.reshape(C, L)
    return np.ascontiguousarray(out.astype(ml_dtypes.bfloat16))


TRACE = False
LAST_EXEC_NS = None
LAST_TRACE_DIR = None


def kernel(**inputs):
    global LAST_EXEC_NS, LAST_TRACE_DIR
    from concourse.bass_utils import run_bass_kernel_spmd

    if "nc" not in _cached:
        _cached["nc"] = _build_program()
    nc = _cached["nc"]

    per_sigma = _host_precompute(inputs)
    x = np.asarray(inputs["x"], np.float32)
    in_maps = []
    for c in range(NCORES):
        b, sg = c // 2, c % 2
        m = dict(per_sigma[sg])
        m["ximgs"] = _pack_images(x[b])
        in_maps.append(m)

    kw = {}
    if TRACE:
        import tempfile
        LAST_TRACE_DIR = tempfile.mkdtemp(prefix="bass_trace_")
        kw = dict(trace=True, tmpdir=LAST_TRACE_DIR)
    r = run_bass_kernel_spmd(nc, in_maps, list(range(NCORES)), **kw)
    if r.exec_time_ns is not None:
        LAST_EXEC_NS = r.exec_time_ns
    res = r.results
    out = np.empty((B, C, H, W), np.float32)
    for b in range(B):
        out[b] = np.asarray(res[2 * b]["out_f"], np.float32).reshape(C, H, W)
    return out
